# revision 14
# baseline (speedup 1.0000x reference)
"""Trainium2 Bass kernel for nn_Directionalmamba (B=8, CH=256, H=W=64).

Sharding: data-parallel over batch — each of the 8 NeuronCores runs one batch
element end-to-end (1x1 conv + BN/ReLU front-end, 4 directional selective
scans, 4 directional 5-tap conv branches, output assembly). No collectives.

Key mappings per core:
  - 1x1 conv / all projections: fp32r matmuls (F=512 chunks, PSUM accum).
  - Directional orders produced via layout tricks: transposed eviction for
    dir1; row-doubled layout for the two diagonal dirs so the (i+j)%64 /
    (j-i)%64 gathers become affine access patterns.
  - Conv branches = 5 shifted-AP matmuls on a zero-padded SBUF tile.
  - Selective scan: hardware prefix scan (tensor_tensor_scan) per (n, chunk)
    on GpSimd; a=exp(dt*A[:,n]) on ACT via per-partition scale; b=u*B with
    B/C broadcast through DMA-replicated (t,n)-interleaved bf16 rows;
    y=sum_n h*C via a masked cumulative scan (segment sum).
"""
import numpy as np

import concourse.bass as bass
import concourse.tile as tile
from concourse import mybir, bacc
from concourse.bass_utils import run_bass_kernel_spmd

F32 = mybir.dt.float32
F32R = mybir.dt.float32r
BF16 = mybir.dt.float16
AOT = mybir.AluOpType
ACTF = mybir.ActivationFunctionType

CH = 256
D, E, N, DTR, KCONV = 64, 128, 16, 4, 4
H = W = 64
L = H * W              # 4096
FC = 512               # matmul moving-dim chunk
NFC = L // FC          # 8
TC = 256               # scan time chunk
NTC = L // TC          # 16
PADW = 68
EPS = 1e-5

_CACHE = {}


def _m_ap(v, dims, extra_offset=0, keep_partition=True):
    """Manual access pattern: replace free dims of AP `v` with `dims`
    ([step, count] pairs, arbitrary steps) at `extra_offset` elements."""
    w = v.copy()
    w.offset = v.offset + extra_offset
    lead = [list(v.ap[0])] if keep_partition else []
    w.ap = mybir.VecI64Pair(lead + [list(d) for d in dims])
    return w


def _build_nc():
    nc = bacc.Bacc("TRN2", target_bir_lowering=False, debug=False)
    ap = {}

    def din(name, shape, dt=F32):
        ap[name] = nc.dram_tensor(name, list(shape), dt, kind="ExternalInput").ap()

    din("xb", (CH, L), F32R)
    din("x2b", (CH, L), F32R)
    din("w1t", (4, 128, 2, 128), F32R)   # [kk][k][m][j]: lhsT for 1x1 (BN-folded)
    din("b1f", (128, 2))
    din("wbr", (2, 128, 20, 64), F32R)   # [kk][k][dir*5+tap][o]
    din("bbr", (64, 4))
    din("winT", (64, 256), F32R)
    din("bin2", (128, 2))
    din("convw", (128, 4))
    din("convb", (128, 1))
    din("nconvb", (128, 1))
    din("wxT", (128, 64), F32R)
    din("wdtT", (4, 128), F32R)
    din("bdt", (128, 1))
    din("acols", (128, 16))
    din("dskip", (128, 1))
    din("woutT", (128, 64), F32R)
    din("bout", (64, 1))

    out_ap = nc.dram_tensor("out", [CH, L], F32, kind="ExternalOutput").ap()
    xc01 = nc.dram_tensor("xc01", [128, L], F32R).ap()
    xcdbl = nc.dram_tensor("xcdbl", [128, 2 * L], F32R).ap()
    cbr = nc.dram_tensor("cbr", [4, 64, L], F32).ap()
    bcp = nc.dram_tensor("bcp", [4, 2, 16 * L], BF16).ap()
    md = nc.dram_tensor("md", [4, 64, 2 * L], F32).ap()

    with tile.TileContext(nc) as tc:
        _body(tc, ap, out_ap, xc01, xcdbl, cbr, bcp, md)
    nc.compile()
    return nc


def _body(tc, ap, out_ap, xc01, xcdbl, cbr, bcp, md):
    nc = tc.nc
    with nc.allow_low_precision(reason="f32r tags are byte-identical to f32"), \
         tc.tile_pool(name="wts", bufs=1) as wpool:
        _body2(tc, wpool, ap, out_ap, xc01, xcdbl, cbr, bcp, md)


def _body2(tc, wpool, ap, out_ap, xc01, xcdbl, cbr, bcp, md):
    nc = tc.nc

    def wtile(name, shape, dt=F32):
        t = wpool.tile(list(shape), dt, name=name)
        nc.sync.dma_start(t[:], ap[name])
        return t

    winT = wtile("winT", (64, 256), F32R)
    bin2 = wtile("bin2", (128, 2))
    convw = wtile("convw", (128, 4))
    convb = wtile("convb", (128, 1))
    nconvb = wtile("nconvb", (128, 1))
    wxT = wtile("wxT", (128, 64), F32R)
    wdtT = wtile("wdtT", (4, 128), F32R)
    bdt = wtile("bdt", (128, 1))
    acols = wtile("acols", (128, 16))
    dskip = wtile("dskip", (128, 1))
    woutT = wtile("woutT", (128, 64), F32R)
    bout = wtile("bout", (64, 1))
    b1f = wtile("b1f", (128, 2))
    bbr = wtile("bbr", (64, 4))

    # ================= PHASE A: 1x1 conv + BN/ReLU + branches =================
    with tc.tile_pool(name="phA", bufs=1) as pa, \
         tc.tile_pool(name="phAp", bufs=3, space="PSUM") as pap:
        w1t = pa.tile([128, 4, 2, 128], F32R)
        nc.sync.dma_start(w1t[:], ap["w1t"].rearrange("a k b m -> k a b m"))
        wbr = pa.tile([128, 2, 20, 64], F32R)
        nc.sync.dma_start(wbr[:], ap["wbr"].rearrange("a k c m -> k a c m"))

        xk = []
        for i, (src, half) in enumerate([("xb", 0), ("xb", 1), ("x2b", 0), ("x2b", 1)]):
            t = pa.tile([128, L], F32R, tag=f"xk{i}", name=f"xk{i}")
            nc.sync.dma_start(t[:], ap[src][128 * half:128 * (half + 1), :])
            xk.append(t)

        pads = [pa.tile([128, PADW * PADW], F32R, tag=f"pad{i}", name=f"pad{i}")
                for i in range(2)]
        nc.vector.memset(pads[0][:].bitcast(F32), 0.0)
        nc.vector.memset(pads[1][:].bitcast(F32), 0.0)
        xc01_sb = pa.tile([128, L], F32R)
        xcdbl_sb = pa.tile([128, 2 * L], F32R)

        for m in range(2):
            for fc in range(NFC):
                ps = pap.tile([128, FC], F32, tag="ps1x1")
                for kk in range(4):
                    nc.tensor.matmul(
                        ps[:], w1t[:, kk, m, :].bitcast(F32R),
                        xk[kk][:, fc * FC:(fc + 1) * FC].bitcast(F32R),
                        start=(kk == 0), stop=(kk == 3))
                ps3 = ps[:].rearrange("p (i j) -> p i j", i=8, j=64)
                padv = pads[m][:].rearrange("p (r c) -> p r c", r=PADW, c=PADW)
                nc.scalar.activation(
                    padv[:, 2 + 8 * fc:2 + 8 * fc + 8, 2:66], ps3,
                    ACTF.Relu, bias=b1f[:, m:m + 1])
                if m == 0:
                    nc.scalar.activation(
                        xc01_sb[0:64, fc * FC:(fc + 1) * FC], ps[0:64, :],
                        ACTF.Relu, bias=b1f[0:64, 0:1])
                    tr = xc01_sb[64:128, :].rearrange(
                        "p (j i) -> p i j", j=64, i=64)[:, 8 * fc:8 * fc + 8, :]
                    nc.scalar.activation(tr, ps3[64:128], ACTF.Relu,
                                         bias=b1f[64:128, 0:1])
                else:
                    dblv = xcdbl_sb[:].rearrange("p (r c) -> p r c", r=64, c=128)
                    nc.scalar.activation(
                        dblv[:, 8 * fc:8 * fc + 8, 0:64], ps3,
                        ACTF.Relu, bias=b1f[:, 1:2])
                    nc.scalar.activation(
                        dblv[:, 8 * fc:8 * fc + 8, 64:128], ps3,
                        ACTF.Relu, bias=b1f[:, 1:2])
        nc.sync.dma_start(xc01, xc01_sb[:])
        nc.sync.dma_start(xcdbl, xcdbl_sb[:])

        # branches: taps (dr, dc): c1 (0,s) c2 (s,0) c3 (s,-s) c4 (s,-s)
        tap_dirs = [(0, 1), (1, 0), (1, -1), (1, -1)]
        cbr_sb = pa.tile([64, L], F32)
        for d in range(4):
            sr, sc = tap_dirs[d]
            for fc in range(NFC):
                psb = pap.tile([64, FC], F32, tag="psbr")
                first = True
                for s in range(-2, 3):
                    dr, dc = sr * s, sc * s
                    for kk in range(2):
                        rhs = pads[kk][:].rearrange(
                            "p (r c) -> p r c", r=PADW, c=PADW)[
                            :, 2 + 8 * fc + dr:2 + 8 * fc + dr + 8,
                            2 + dc:2 + dc + 64]
                        nc.tensor.matmul(
                            psb[:], wbr[:, kk, d * 5 + s + 2, :].bitcast(F32R),
                            rhs.bitcast(F32R),
                            start=first, stop=(s == 2 and kk == 1))
                        first = False
                nc.scalar.activation(
                    cbr_sb[:, fc * FC:(fc + 1) * FC], psb[:],
                    ACTF.Identity, bias=bbr[:, d:d + 1])
            nc.sync.dma_start(cbr[d], cbr_sb[:])

    # ================= PHASE B: 4 directional mamba sequences =================
    with tc.tile_pool(name="phB", bufs=1) as pb, \
         tc.tile_pool(name="phBr", bufs=2) as pbr, \
         tc.tile_pool(name="phBs", bufs=2) as pbs, \
         tc.tile_pool(name="phBp", bufs=4, space="PSUM") as pbp:
        for d in range(4):
            xi_pad = pb.tile([128, L + 32], F32R, tag="xi_pad")
            zs = pb.tile([128, L], F32, tag="zs")
            dt = pb.tile([128, L], F32, tag="dt")
            u = pb.tile([128, L], F32R, tag="u")
            dbl_sb = pb.tile([128, L], F32R, tag="dbl")
            bc16 = pb.tile([64, L], BF16, tag="bc16")
            b_all = pb.tile([128, 16 * TC], F32, tag="b_o_shared")

            nc.vector.memset(xi_pad[:, 0:3].bitcast(F32), 0.0)

            # ---- in-proj ----
            for fc in range(NFC):
                rr = pbr.tile([64, FC], F32R, tag="rhs")
                if d == 0:
                    nc.sync.dma_start(rr[:], xc01[0:64, fc * FC:(fc + 1) * FC])
                elif d == 1:
                    nc.sync.dma_start(rr[:], xc01[64:128, fc * FC:(fc + 1) * FC])
                elif d == 2:
                    src = _m_ap(xcdbl[0:64, :], [[129, 8], [1, 64]], 129 * 8 * fc)
                    nc.sync.dma_start(rr[:], src)
                else:
                    src = _m_ap(xcdbl[64:128, :], [[127, 8], [1, 64]],
                                64 + 127 * 8 * fc)
                    nc.sync.dma_start(rr[:], src)
                pxi = pbp.tile([128, FC], F32, tag="psB", name="pxi")
                nc.tensor.matmul(pxi[:], winT[:, 0:128].bitcast(F32R),
                                 rr[:].bitcast(F32R), start=True, stop=True)
                nc.scalar.activation(xi_pad[:, 3 + fc * FC:3 + (fc + 1) * FC],
                                     pxi[:], ACTF.Identity, bias=bin2[:, 0:1])
                pz = pbp.tile([128, FC], F32, tag="psB", name="pz")
                nc.tensor.matmul(pz[:], winT[:, 128:256].bitcast(F32R),
                                 rr[:].bitcast(F32R), start=True, stop=True)
                nc.scalar.activation(zs[:, fc * FC:(fc + 1) * FC], pz[:],
                                     ACTF.Identity, bias=bin2[:, 1:2])

            # ---- causal depthwise conv1d + SiLU -> u holds xic ----
            nc.vector.tensor_scalar(u[:], xi_pad[:, 0:L], convw[:, 0:1],
                                    None, AOT.mult)
            for k in range(1, 4):
                nc.vector.scalar_tensor_tensor(
                    u[:], xi_pad[:, k:k + L], convw[:, k:k + 1], u[:],
                    AOT.mult, AOT.add)
            # silu(u + convb) = (u+convb) * 1/(1+exp(-(u+convb)))
            nc.scalar.activation(dt[:], u[:], ACTF.Exp, scale=-1.0,
                                 bias=nconvb[:, 0:1])
            nc.vector.tensor_scalar(dt[:], dt[:], 1.0, None, AOT.add)
            nc.vector.reciprocal(dt[:], dt[:])
            nc.vector.scalar_tensor_tensor(u[:], u[:], convb[:, 0:1], dt[:],
                                           AOT.add, AOT.mult)

            # ---- dbl projection (dt_raw | B | C), dt projection ----
            for fc in range(NFC):
                pdb = pbp.tile([64, FC], F32, tag="psB", name="pdb")
                nc.tensor.matmul(pdb[:], wxT[:].bitcast(F32R),
                                 u[:, fc * FC:(fc + 1) * FC].bitcast(F32R),
                                 start=True, stop=True)
                nc.scalar.activation(dbl_sb[0:64, fc * FC:(fc + 1) * FC],
                                     pdb[:], ACTF.Copy)
            for fc in range(NFC):
                pdt = pbp.tile([128, FC], F32, tag="psB", name="pdt")
                nc.tensor.matmul(pdt[:], wdtT[:].bitcast(F32R),
                                 dbl_sb[0:4, fc * FC:(fc + 1) * FC].bitcast(F32R),
                                 start=True, stop=True)
                nc.scalar.activation(dt[:, fc * FC:(fc + 1) * FC], pdt[:],
                                     ACTF.Exp, bias=bdt[:, 0:1])
                nc.scalar.activation(dt[:, fc * FC:(fc + 1) * FC],
                                     dt[:, fc * FC:(fc + 1) * FC],
                                     ACTF.Ln, bias=1.0)

            # g = (xic * Dskip) * zs stored into xi_pad[:, 0:L] (xi_pad dead)
            nc.vector.scalar_tensor_tensor(xi_pad[:, 0:L], u[:], dskip[:, 0:1],
                                           zs[:], AOT.mult, AOT.mult)
            # u := dt * xic (in place; after dbl used xic)
            nc.vector.tensor_tensor(u[:], u[:], dt[:], AOT.mult)

            # export B,C as bf16 (t,n)-interleaved rows to DRAM
            nc.scalar.activation(bc16[32:64, :], dbl_sb[32:64, :], ACTF.Copy)
            for bi in range(2):
                dst = _m_ap(bcp[d, bi, :], [[1, 16], [16, L]],
                            keep_partition=False)
                nc.sync.dma_start(dst, bc16[32 + 16 * bi:48 + 16 * bi, :])

            # zs := silu(zs) using exp/recip (dbl_sb rows free as scratch)
            nc.scalar.activation(dbl_sb[:], zs[:], ACTF.Exp, scale=-1.0)
            nc.vector.tensor_scalar(dbl_sb[:], dbl_sb[:], 1.0, None, AOT.add)
            nc.vector.reciprocal(dbl_sb[:], dbl_sb[:])
            nc.gpsimd.tensor_tensor(zs[:], zs[:], dbl_sb[:], AOT.mult)

            # ---- scan chunks (software-pipelined y-stage) ----
            h_prev = None
            c_prev = None
            for c in range(NTC + 1):
                if c < NTC:
                    brep = pbs.tile([128, 16 * TC], BF16, tag="brep")
                    nc.sync.dma_start(
                        brep[:], bcp[d, 0, 16 * TC * c:16 * TC * (c + 1)]
                        .partition_broadcast(128))
                    crep = pbs.tile([128, 16 * TC], BF16, tag="crep")
                    nc.sync.dma_start(
                        crep[:], bcp[d, 1, 16 * TC * c:16 * TC * (c + 1)]
                        .partition_broadcast(128))
                    u_bc = _m_ap(u[:], [[1, TC], [0, 16]], TC * c)
                    nc.gpsimd.tensor_tensor(
                        b_all[:].rearrange("p (t n) -> p t n", n=16),
                        u_bc, brep[:].rearrange("p (t n) -> p t n", n=16),
                        AOT.mult)
                    h_all = pbs.tile([128, 16 * TC], F32, tag="h_all")
                    hv = h_all[:].rearrange("p (t n) -> p t n", n=16)
                    bv = b_all[:].rearrange("p (t n) -> p t n", n=16)
                    for n in range(16):
                        a_n = pbs.tile([128, TC], F32, tag="a_n")
                        nc.scalar.activation(a_n[:], dt[:, TC * c:TC * (c + 1)],
                                             ACTF.Exp, scale=acols[:, n:n + 1])
                        if c == 0:
                            init = 0.0
                        else:
                            init = h_prev[:].rearrange(
                                "p (t n) -> p t n", n=16)[:, TC - 1:TC, n]
                        nc.vector.tensor_tensor_scan(
                            hv[:, :, n], a_n[:], bv[:, :, n], init,
                            AOT.mult, AOT.add)
                if c > 0:
                    cm1 = c - 1
                    eng = nc.vector if (cm1 % 2 == 0) else nc.gpsimd
                    eng.tensor_tensor(h_prev[:], h_prev[:], c_prev[:], AOT.mult)
                    pv = h_prev[:].rearrange("p (t n) -> p t n", n=16)
                    for half in (8, 4, 2, 1):
                        nc.gpsimd.tensor_tensor(pv[:, :, 0:half],
                                                pv[:, :, 0:half],
                                                pv[:, :, half:2 * half], AOT.add)
                    tmp = pbs.tile([128, TC], F32, tag="tmp")
                    nc.vector.tensor_tensor(tmp[:], pv[:, :, 0],
                                            zs[:, TC * cm1:TC * c], AOT.mult)
                    nc.gpsimd.tensor_tensor(
                        xi_pad[:, TC * cm1:TC * c], tmp[:],
                        xi_pad[:, TC * cm1:TC * c], AOT.add)
                if c < NTC:
                    h_prev = h_all
                    c_prev = crep

            # ---- out-proj ----
            o_sb = pb.tile([64, L], F32, tag="b_o_shared")
            for fc in range(NFC):
                po = pbp.tile([64, FC], F32, tag="psB", name="po")
                nc.tensor.matmul(po[:], woutT[:].bitcast(F32R),
                                 xi_pad[:, fc * FC:(fc + 1) * FC].bitcast(F32R),
                                 start=True, stop=True)
                nc.scalar.activation(o_sb[:, fc * FC:(fc + 1) * FC], po[:],
                                     ACTF.Identity, bias=bout[:, 0:1])
            nc.sync.dma_start(md[d][:, 0:L], o_sb[:])
            if d >= 2:
                nc.sync.dma_start(md[d][:, L:2 * L], o_sb[:])

    # ================= PHASE C: assembly =================
    with tc.tile_pool(name="phC", bufs=2) as pc:
        for d in range(4):
            mo = pc.tile([64, 2 * L], F32, tag="mo")
            if d < 2:
                nc.sync.dma_start(mo[:, 0:L], md[d][:, 0:L])
            else:
                nc.sync.dma_start(mo[:], md[d])
            cb = pc.tile([64, L], F32, tag="cb")
            nc.sync.dma_start(cb[:], cbr[d])
            ofin = pc.tile([64, L], F32, tag="ofin")
            if d == 0:
                src = mo[:, 0:L]
            elif d == 1:
                src = _m_ap(mo[:], [[1, 64], [64, 64]])
            elif d == 2:
                src = _m_ap(mo[:], [[-63, 64], [64, 64]], L)
            else:
                src = _m_ap(mo[:], [[65, 64], [64, 64]])
            nc.vector.tensor_tensor(ofin[:], src, cb[:], AOT.add)
            nc.sync.dma_start(out_ap[64 * d:64 * (d + 1), :], ofin[:])




def _wxt64(Wx):
    wt = np.asarray(Wx).T.astype(np.float32)  # (128, 36)
    out = np.zeros((128, 64), np.float32)
    out[:, 0:4] = wt[:, 0:4]
    out[:, 32:48] = wt[:, 4:20]
    out[:, 48:64] = wt[:, 20:36]
    return out

def _prep_weights(w1, b1, bn_g, bn_b, bn_m, bn_v,
                  hconv_w, hconv_b, wconv_w, wconv_b, d19_w, d19_b, d37_w,
                  d37_b, Win, bin_, convw, convb, Wx, Wdt, bdt, Alog, Dskip,
                  Wout, bout):
    f32 = np.float32
    scale = (bn_g / np.sqrt(bn_v + EPS)).astype(f32)
    w1f = (np.asarray(w1)[:, :, 0, 0] * scale[:, None]).astype(f32)  # (256, 512)
    b1fv = ((np.asarray(b1) - bn_m) * scale + bn_b).astype(f32)

    w1t = np.zeros((4, 128, 2, 128), f32)
    for kk in range(4):
        for m in range(2):
            w1t[kk, :, m, :] = w1f[m * 128:(m + 1) * 128,
                                   kk * 128:(kk + 1) * 128].T
    b1f = np.stack([b1fv[0:128], b1fv[128:256]], axis=1)

    # branch taps: weight[s] for offset pattern (see _body tap_dirs)
    taps = np.zeros((4, 5, 64, 256), f32)
    for s in range(-2, 3):
        taps[0, s + 2] = np.asarray(hconv_w)[:, :, 0, s + 2]
        taps[1, s + 2] = np.asarray(wconv_w)[:, :, s + 2, 0]
        taps[2, s + 2] = np.asarray(d19_w)[:, :, s + 2, 0]
        taps[3, s + 2] = np.asarray(d37_w)[:, :, 0, 2 - s]
    wbr = np.zeros((2, 128, 20, 64), f32)
    for kk in range(2):
        for idx in range(20):
            dd, ss = idx // 5, idx % 5
            wbr[kk, :, idx, :] = taps[dd, ss, :, kk * 128:(kk + 1) * 128].T
    bbr = np.stack([hconv_b, wconv_b, d19_b, d37_b], axis=1).astype(f32)

    return dict(
        w1t=w1t, b1f=b1f, wbr=wbr, bbr=bbr,
        winT=np.asarray(Win).T.astype(f32).copy(),
        bin2=np.stack([bin_[0:128], bin_[128:256]], axis=1).astype(f32),
        convw=np.asarray(convw)[:, 0, :].astype(f32).copy(),
        convb=np.asarray(convb).reshape(128, 1).astype(f32),
        nconvb=(-np.asarray(convb).reshape(128, 1)).astype(f32),
        wxT=_wxt64(Wx),
        wdtT=np.asarray(Wdt).T.astype(f32).copy(),
        bdt=np.asarray(bdt).reshape(128, 1).astype(f32),
        acols=(-np.exp(np.asarray(Alog))).astype(f32),
        dskip=np.asarray(Dskip).reshape(128, 1).astype(f32),
        woutT=np.asarray(Wout).T.astype(f32).copy(),
        bout=np.asarray(bout).reshape(64, 1).astype(f32),
    )


def get_nc():
    if "nc" not in _CACHE:
        _CACHE["nc"] = _build_nc()
    return _CACHE["nc"]


def kernel(x, x2, **kw):
    nc = get_nc()
    wts = _prep_weights(**kw)
    xf = np.asarray(x, np.float32).reshape(8, CH, L)
    x2f = np.asarray(x2, np.float32).reshape(8, CH, L)
    in_maps = []
    for b in range(8):
        m = dict(wts)
        m["xb"] = np.ascontiguousarray(xf[b])
        m["x2b"] = np.ascontiguousarray(x2f[b])
        in_maps.append(m)
    res = run_bass_kernel_spmd(nc, in_maps, core_ids=list(range(8)))
    out = np.stack([res.results[b]["out"] for b in range(8)], axis=0)
    return out.reshape(8, CH, H, W).astype(np.float32)


# revision 15
# speedup vs baseline: 1.2737x; 1.2737x over previous
"""Trainium2 Bass kernel for nn_Directionalmamba (B=8, CH=256, H=W=64).

Sharding: data-parallel over batch — each of the 8 NeuronCores runs one batch
element end-to-end (1x1 conv + BN/ReLU front-end, 4 directional selective
scans, 4 directional 5-tap conv branches, output assembly). No collectives.

Key mappings per core:
  - 1x1 conv / all projections: fp32r matmuls (F=512 chunks, PSUM accum).
  - Directional orders produced via layout tricks: transposed eviction for
    dir1; row-doubled layout for the two diagonal dirs so the (i+j)%64 /
    (j-i)%64 gathers become affine access patterns.
  - Conv branches = 5 shifted-AP matmuls on a zero-padded SBUF tile.
  - Selective scan: hardware prefix scan (tensor_tensor_scan) per (n, chunk)
    on GpSimd; a=exp(dt*A[:,n]) on ACT via per-partition scale; b=u*B with
    B/C broadcast through DMA-replicated (t,n)-interleaved bf16 rows;
    y=sum_n h*C via a masked cumulative scan (segment sum).
"""
import numpy as np

import concourse.bass as bass
import concourse.tile as tile
from concourse import mybir, bacc
from concourse.bass_utils import run_bass_kernel_spmd

F32 = mybir.dt.float32
F32R = mybir.dt.float32r
BF16 = mybir.dt.float16
AOT = mybir.AluOpType
ACTF = mybir.ActivationFunctionType

CH = 256
D, E, N, DTR, KCONV = 64, 128, 16, 4, 4
H = W = 64
L = H * W              # 4096
FC = 512               # matmul moving-dim chunk
NFC = L // FC          # 8
TC = 256               # scan time chunk
NTC = L // TC          # 16
PADW = 68
EPS = 1e-5

_CACHE = {}


def _m_ap(v, dims, extra_offset=0, keep_partition=True):
    """Manual access pattern: replace free dims of AP `v` with `dims`
    ([step, count] pairs, arbitrary steps) at `extra_offset` elements."""
    w = v.copy()
    w.offset = v.offset + extra_offset
    lead = [list(v.ap[0])] if keep_partition else []
    w.ap = mybir.VecI64Pair(lead + [list(d) for d in dims])
    return w


def _build_nc():
    nc = bacc.Bacc("TRN2", target_bir_lowering=False, debug=False)
    ap = {}

    def din(name, shape, dt=F32):
        ap[name] = nc.dram_tensor(name, list(shape), dt, kind="ExternalInput").ap()

    din("xb", (CH, L), F32R)
    din("x2b", (CH, L), F32R)
    din("w1t", (4, 128, 2, 128), F32R)   # [kk][k][m][j]: lhsT for 1x1 (BN-folded)
    din("b1f", (128, 2))
    din("wbr", (2, 128, 20, 64), F32R)   # [kk][k][dir*5+tap][o]
    din("bbr", (64, 4))
    din("winT", (64, 256), F32R)
    din("bin2", (128, 2))
    din("convw", (128, 4))
    din("convb", (128, 1))
    din("nconvb", (128, 1))
    din("wxT", (128, 64), F32R)
    din("wdtT", (4, 128), F32R)
    din("bdt", (128, 1))
    din("acols", (128, 16))
    din("dskip", (128, 1))
    din("woutT", (128, 64), F32R)
    din("bout", (64, 1))

    out_ap = nc.dram_tensor("out", [CH, L], F32, kind="ExternalOutput").ap()
    xc01 = nc.dram_tensor("xc01", [128, L], F32R).ap()
    xcdbl = nc.dram_tensor("xcdbl", [128, 2 * L], F32R).ap()
    cbr = nc.dram_tensor("cbr", [4, 64, L], F32).ap()
    bcp = nc.dram_tensor("bcp", [4, 2, 16 * L], BF16).ap()
    md = nc.dram_tensor("md", [4, 64, 2 * L], F32).ap()

    with tile.TileContext(nc) as tc:
        _body(tc, ap, out_ap, xc01, xcdbl, cbr, bcp, md)
    nc.compile()
    return nc


def _body(tc, ap, out_ap, xc01, xcdbl, cbr, bcp, md):
    nc = tc.nc
    with nc.allow_low_precision(reason="f32r tags are byte-identical to f32"), \
         tc.tile_pool(name="wts", bufs=1) as wpool:
        _body2(tc, wpool, ap, out_ap, xc01, xcdbl, cbr, bcp, md)


def _body2(tc, wpool, ap, out_ap, xc01, xcdbl, cbr, bcp, md):
    nc = tc.nc

    def wtile(name, shape, dt=F32):
        t = wpool.tile(list(shape), dt, name=name)
        nc.sync.dma_start(t[:], ap[name])
        return t

    winT = wtile("winT", (64, 256), F32R)
    bin2 = wtile("bin2", (128, 2))
    convw = wtile("convw", (128, 4))
    convb = wtile("convb", (128, 1))
    nconvb = wtile("nconvb", (128, 1))
    wxT = wtile("wxT", (128, 64), F32R)
    wdtT = wtile("wdtT", (4, 128), F32R)
    bdt = wtile("bdt", (128, 1))
    acols = wtile("acols", (128, 16))
    dskip = wtile("dskip", (128, 1))
    woutT = wtile("woutT", (128, 64), F32R)
    bout = wtile("bout", (64, 1))
    b1f = wtile("b1f", (128, 2))
    bbr = wtile("bbr", (64, 4))

    # ================= PHASE A: 1x1 conv + BN/ReLU + branches =================
    with tc.tile_pool(name="phA", bufs=1) as pa, \
         tc.tile_pool(name="phAp", bufs=3, space="PSUM") as pap:
        w1t = pa.tile([128, 4, 2, 128], F32R)
        nc.sync.dma_start(w1t[:], ap["w1t"].rearrange("a k b m -> k a b m"))
        wbr = pa.tile([128, 2, 20, 64], F32R)
        nc.sync.dma_start(wbr[:], ap["wbr"].rearrange("a k c m -> k a c m"))

        xk = []
        for i, (src, half) in enumerate([("xb", 0), ("xb", 1), ("x2b", 0), ("x2b", 1)]):
            t = pa.tile([128, L], F32R, tag=f"xk{i}", name=f"xk{i}")
            nc.sync.dma_start(t[:], ap[src][128 * half:128 * (half + 1), :])
            xk.append(t)

        pads = [pa.tile([128, PADW * PADW], F32R, tag=f"pad{i}", name=f"pad{i}")
                for i in range(2)]
        nc.vector.memset(pads[0][:].bitcast(F32), 0.0)
        nc.vector.memset(pads[1][:].bitcast(F32), 0.0)
        xc01_sb = pa.tile([128, L], F32R)
        xcdbl_sb = pa.tile([128, 2 * L], F32R)

        for m in range(2):
            for fc in range(NFC):
                ps = pap.tile([128, FC], F32, tag="ps1x1")
                for kk in range(4):
                    nc.tensor.matmul(
                        ps[:], w1t[:, kk, m, :].bitcast(F32R),
                        xk[kk][:, fc * FC:(fc + 1) * FC].bitcast(F32R),
                        start=(kk == 0), stop=(kk == 3))
                ps3 = ps[:].rearrange("p (i j) -> p i j", i=8, j=64)
                padv = pads[m][:].rearrange("p (r c) -> p r c", r=PADW, c=PADW)
                nc.scalar.activation(
                    padv[:, 2 + 8 * fc:2 + 8 * fc + 8, 2:66], ps3,
                    ACTF.Relu, bias=b1f[:, m:m + 1])
                if m == 0:
                    nc.scalar.activation(
                        xc01_sb[0:64, fc * FC:(fc + 1) * FC], ps[0:64, :],
                        ACTF.Relu, bias=b1f[0:64, 0:1])
                    tr = xc01_sb[64:128, :].rearrange(
                        "p (j i) -> p i j", j=64, i=64)[:, 8 * fc:8 * fc + 8, :]
                    nc.scalar.activation(tr, ps3[64:128], ACTF.Relu,
                                         bias=b1f[64:128, 0:1])
                else:
                    dblv = xcdbl_sb[:].rearrange("p (r c) -> p r c", r=64, c=128)
                    nc.scalar.activation(
                        dblv[:, 8 * fc:8 * fc + 8, 0:64], ps3,
                        ACTF.Relu, bias=b1f[:, 1:2])
                    nc.scalar.activation(
                        dblv[:, 8 * fc:8 * fc + 8, 64:128], ps3,
                        ACTF.Relu, bias=b1f[:, 1:2])
        nc.sync.dma_start(xc01, xc01_sb[:])
        nc.sync.dma_start(xcdbl, xcdbl_sb[:])

        # branches: taps (dr, dc): c1 (0,s) c2 (s,0) c3 (s,-s) c4 (s,-s)
        tap_dirs = [(0, 1), (1, 0), (1, -1), (1, -1)]
        cbr_sb = pa.tile([64, L], F32)
        for d in range(4):
            sr, sc = tap_dirs[d]
            for fc in range(NFC):
                psb = pap.tile([64, FC], F32, tag="psbr")
                first = True
                for s in range(-2, 3):
                    dr, dc = sr * s, sc * s
                    for kk in range(2):
                        rhs = pads[kk][:].rearrange(
                            "p (r c) -> p r c", r=PADW, c=PADW)[
                            :, 2 + 8 * fc + dr:2 + 8 * fc + dr + 8,
                            2 + dc:2 + dc + 64]
                        nc.tensor.matmul(
                            psb[:], wbr[:, kk, d * 5 + s + 2, :].bitcast(F32R),
                            rhs.bitcast(F32R),
                            start=first, stop=(s == 2 and kk == 1))
                        first = False
                nc.scalar.activation(
                    cbr_sb[:, fc * FC:(fc + 1) * FC], psb[:],
                    ACTF.Identity, bias=bbr[:, d:d + 1])
            nc.sync.dma_start(cbr[d], cbr_sb[:])

    # ================= PHASE B: 4 directional mamba sequences =================
    with tc.tile_pool(name="phB", bufs=1) as pb, \
         tc.tile_pool(name="phBr", bufs=2) as pbr, \
         tc.tile_pool(name="phBs", bufs=2) as pbs, \
         tc.tile_pool(name="phBp", bufs=4, space="PSUM") as pbp:
        for d in range(4):
            xi_pad = pb.tile([128, L + 32], F32R, tag="xi_pad")
            zs = pb.tile([128, L], F32, tag="zs")
            dt = pb.tile([128, L], F32, tag="dt")
            u = pb.tile([128, L], F32R, tag="u")
            dbl_sb = pb.tile([128, L], F32R, tag="dbl")
            bc16 = pb.tile([64, L], BF16, tag="bc16")
            b_all = pb.tile([128, 16 * TC], F32, tag="b_o_shared")

            nc.vector.memset(xi_pad[:, 0:3].bitcast(F32), 0.0)

            # ---- in-proj ----
            for fc in range(NFC):
                rr = pbr.tile([64, FC], F32R, tag="rhs")
                if d == 0:
                    nc.sync.dma_start(rr[:], xc01[0:64, fc * FC:(fc + 1) * FC])
                elif d == 1:
                    nc.sync.dma_start(rr[:], xc01[64:128, fc * FC:(fc + 1) * FC])
                elif d == 2:
                    src = _m_ap(xcdbl[0:64, :], [[129, 8], [1, 64]], 129 * 8 * fc)
                    nc.sync.dma_start(rr[:], src)
                else:
                    src = _m_ap(xcdbl[64:128, :], [[127, 8], [1, 64]],
                                64 + 127 * 8 * fc)
                    nc.sync.dma_start(rr[:], src)
                pxi = pbp.tile([128, FC], F32, tag="psB", name="pxi")
                nc.tensor.matmul(pxi[:], winT[:, 0:128].bitcast(F32R),
                                 rr[:].bitcast(F32R), start=True, stop=True)
                nc.scalar.activation(xi_pad[:, 3 + fc * FC:3 + (fc + 1) * FC],
                                     pxi[:], ACTF.Identity, bias=bin2[:, 0:1])
                pz = pbp.tile([128, FC], F32, tag="psB", name="pz")
                nc.tensor.matmul(pz[:], winT[:, 128:256].bitcast(F32R),
                                 rr[:].bitcast(F32R), start=True, stop=True)
                nc.scalar.activation(zs[:, fc * FC:(fc + 1) * FC], pz[:],
                                     ACTF.Identity, bias=bin2[:, 1:2])

            # ---- causal depthwise conv1d + SiLU -> u holds xic ----
            nc.vector.tensor_scalar(u[:], xi_pad[:, 0:L], convw[:, 0:1],
                                    None, AOT.mult)
            for k in range(1, 4):
                nc.vector.scalar_tensor_tensor(
                    u[:], xi_pad[:, k:k + L], convw[:, k:k + 1], u[:],
                    AOT.mult, AOT.add)
            # silu(u + convb) = (u+convb) * 1/(1+exp(-(u+convb)))
            nc.scalar.activation(dt[:], u[:], ACTF.Exp, scale=-1.0,
                                 bias=nconvb[:, 0:1])
            nc.vector.tensor_scalar(dt[:], dt[:], 1.0, None, AOT.add)
            nc.vector.reciprocal(dt[:], dt[:])
            nc.vector.scalar_tensor_tensor(u[:], u[:], convb[:, 0:1], dt[:],
                                           AOT.add, AOT.mult)

            # ---- dbl projection (dt_raw | B | C), dt projection ----
            for fc in range(NFC):
                pdb = pbp.tile([64, FC], F32, tag="psB", name="pdb")
                nc.tensor.matmul(pdb[:], wxT[:].bitcast(F32R),
                                 u[:, fc * FC:(fc + 1) * FC].bitcast(F32R),
                                 start=True, stop=True)
                nc.scalar.activation(dbl_sb[0:64, fc * FC:(fc + 1) * FC],
                                     pdb[:], ACTF.Copy)
            for fc in range(NFC):
                pdt = pbp.tile([128, FC], F32, tag="psB", name="pdt")
                nc.tensor.matmul(pdt[:], wdtT[:].bitcast(F32R),
                                 dbl_sb[0:4, fc * FC:(fc + 1) * FC].bitcast(F32R),
                                 start=True, stop=True)
                nc.scalar.activation(dt[:, fc * FC:(fc + 1) * FC], pdt[:],
                                     ACTF.Exp, bias=bdt[:, 0:1])
                nc.scalar.activation(dt[:, fc * FC:(fc + 1) * FC],
                                     dt[:, fc * FC:(fc + 1) * FC],
                                     ACTF.Ln, bias=1.0)

            # g = (xic * Dskip) * zs stored into xi_pad[:, 0:L] (xi_pad dead)
            nc.vector.scalar_tensor_tensor(xi_pad[:, 0:L], u[:], dskip[:, 0:1],
                                           zs[:], AOT.mult, AOT.mult)
            # u := dt * xic (in place; after dbl used xic)
            nc.vector.tensor_tensor(u[:], u[:], dt[:], AOT.mult)

            # export B,C as bf16 (t,n)-interleaved rows to DRAM
            nc.scalar.activation(bc16[32:64, :], dbl_sb[32:64, :], ACTF.Copy)
            for bi in range(2):
                dst = _m_ap(bcp[d, bi, :], [[1, 16], [16, L]],
                            keep_partition=False)
                nc.sync.dma_start(dst, bc16[32 + 16 * bi:48 + 16 * bi, :])

            # zs := silu(zs) using exp/recip (dbl_sb rows free as scratch)
            nc.scalar.activation(dbl_sb[:], zs[:], ACTF.Exp, scale=-1.0)
            nc.vector.tensor_scalar(dbl_sb[:], dbl_sb[:], 1.0, None, AOT.add)
            nc.vector.reciprocal(dbl_sb[:], dbl_sb[:])
            nc.gpsimd.tensor_tensor(zs[:], zs[:], dbl_sb[:], AOT.mult)

            # ---- scan chunks (software-pipelined y-stage) ----
            h_prev = None
            c_prev = None
            for c in range(NTC + 1):
                if c < NTC:
                    brep = pbs.tile([128, 16 * TC], BF16, tag="brep")
                    nc.sync.dma_start(
                        brep[:], bcp[d, 0, 16 * TC * c:16 * TC * (c + 1)]
                        .partition_broadcast(128))
                    crep = pbs.tile([128, 16 * TC], BF16, tag="crep")
                    nc.sync.dma_start(
                        crep[:], bcp[d, 1, 16 * TC * c:16 * TC * (c + 1)]
                        .partition_broadcast(128))
                    u_bc = _m_ap(u[:], [[1, TC], [0, 16]], TC * c)
                    nc.gpsimd.tensor_tensor(
                        b_all[:].rearrange("p (t n) -> p t n", n=16),
                        u_bc, brep[:].rearrange("p (t n) -> p t n", n=16),
                        AOT.mult)
                    h_all = pbs.tile([128, 16 * TC], F32, tag="h_all")
                    hv = h_all[:].rearrange("p (t n) -> p t n", n=16)
                    bv = b_all[:].rearrange("p (t n) -> p t n", n=16)
                    for n in range(16):
                        a_n = pbs.tile([128, TC], F32, tag="a_n")
                        nc.scalar.activation(a_n[:], dt[:, TC * c:TC * (c + 1)],
                                             ACTF.Exp, scale=acols[:, n:n + 1])
                        if c == 0:
                            init = 0.0
                        else:
                            init = h_prev[:].rearrange(
                                "p (t n) -> p t n", n=16)[:, TC - 1:TC, n]
                        nc.vector.tensor_tensor_scan(
                            hv[:, :, n], a_n[:], bv[:, :, n], init,
                            AOT.mult, AOT.add)
                if c > 0:
                    cm1 = c - 1
                    eng = nc.vector if (cm1 % 2 == 0) else nc.gpsimd
                    eng.tensor_tensor(h_prev[:], h_prev[:], c_prev[:], AOT.mult)
                    pv = h_prev[:].rearrange("p (t n) -> p t n", n=16)
                    for half in (8, 4, 2, 1):
                        nc.gpsimd.tensor_tensor(pv[:, :, 0:half],
                                                pv[:, :, 0:half],
                                                pv[:, :, half:2 * half], AOT.add)
                    tmp = pbs.tile([128, TC], F32, tag="tmp")
                    nc.vector.tensor_tensor(tmp[:], pv[:, :, 0],
                                            zs[:, TC * cm1:TC * c], AOT.mult)
                    nc.gpsimd.tensor_tensor(
                        xi_pad[:, TC * cm1:TC * c], tmp[:],
                        xi_pad[:, TC * cm1:TC * c], AOT.add)
                if c < NTC:
                    h_prev = h_all
                    c_prev = crep

            # ---- out-proj ----
            o_sb = pb.tile([64, L], F32, tag="b_o_shared")
            for fc in range(NFC):
                po = pbp.tile([64, FC], F32, tag="psB", name="po")
                nc.tensor.matmul(po[:], woutT[:].bitcast(F32R),
                                 xi_pad[:, fc * FC:(fc + 1) * FC].bitcast(F32R),
                                 start=True, stop=True)
                nc.scalar.activation(o_sb[:, fc * FC:(fc + 1) * FC], po[:],
                                     ACTF.Identity, bias=bout[:, 0:1])
            nc.sync.dma_start(md[d][:, 0:L], o_sb[:])
            if d >= 2:
                nc.sync.dma_start(md[d][:, L:2 * L], o_sb[:])

    # ================= PHASE C: assembly =================
    with tc.tile_pool(name="phC", bufs=2) as pc:
        for d in range(4):
            mo = pc.tile([64, 2 * L], F32, tag="mo")
            if d < 2:
                nc.sync.dma_start(mo[:, 0:L], md[d][:, 0:L])
            else:
                nc.sync.dma_start(mo[:], md[d])
            cb = pc.tile([64, L], F32, tag="cb")
            nc.sync.dma_start(cb[:], cbr[d])
            ofin = pc.tile([64, L], F32, tag="ofin")
            if d == 0:
                src = mo[:, 0:L]
            elif d == 1:
                src = _m_ap(mo[:], [[1, 64], [64, 64]])
            elif d == 2:
                src = _m_ap(mo[:], [[-63, 64], [64, 64]], L)
            else:
                src = _m_ap(mo[:], [[65, 64], [64, 64]])
            nc.vector.tensor_tensor(ofin[:], src, cb[:], AOT.add)
            nc.sync.dma_start(out_ap[64 * d:64 * (d + 1), :], ofin[:])




def _wxt64(Wx):
    wt = np.asarray(Wx).T.astype(np.float32)  # (128, 36)
    out = np.zeros((128, 64), np.float32)
    out[:, 0:4] = wt[:, 0:4]
    out[:, 32:48] = wt[:, 4:20]
    out[:, 48:64] = wt[:, 20:36]
    return out

def _prep_weights(w1, b1, bn_g, bn_b, bn_m, bn_v,
                  hconv_w, hconv_b, wconv_w, wconv_b, d19_w, d19_b, d37_w,
                  d37_b, Win, bin_, convw, convb, Wx, Wdt, bdt, Alog, Dskip,
                  Wout, bout):
    f32 = np.float32
    scale = (bn_g / np.sqrt(bn_v + EPS)).astype(f32)
    w1f = (np.asarray(w1)[:, :, 0, 0] * scale[:, None]).astype(f32)  # (256, 512)
    b1fv = ((np.asarray(b1) - bn_m) * scale + bn_b).astype(f32)

    w1t = np.zeros((4, 128, 2, 128), f32)
    for kk in range(4):
        for m in range(2):
            w1t[kk, :, m, :] = w1f[m * 128:(m + 1) * 128,
                                   kk * 128:(kk + 1) * 128].T
    b1f = np.stack([b1fv[0:128], b1fv[128:256]], axis=1)

    # branch taps: weight[s] for offset pattern (see _body tap_dirs)
    taps = np.zeros((4, 5, 64, 256), f32)
    for s in range(-2, 3):
        taps[0, s + 2] = np.asarray(hconv_w)[:, :, 0, s + 2]
        taps[1, s + 2] = np.asarray(wconv_w)[:, :, s + 2, 0]
        taps[2, s + 2] = np.asarray(d19_w)[:, :, s + 2, 0]
        taps[3, s + 2] = np.asarray(d37_w)[:, :, 0, 2 - s]
    wbr = np.zeros((2, 128, 20, 64), f32)
    for kk in range(2):
        for idx in range(20):
            dd, ss = idx // 5, idx % 5
            wbr[kk, :, idx, :] = taps[dd, ss, :, kk * 128:(kk + 1) * 128].T
    bbr = np.stack([hconv_b, wconv_b, d19_b, d37_b], axis=1).astype(f32)

    return dict(
        w1t=w1t, b1f=b1f, wbr=wbr, bbr=bbr,
        winT=np.asarray(Win).T.astype(f32).copy(),
        bin2=np.stack([bin_[0:128], bin_[128:256]], axis=1).astype(f32),
        convw=np.asarray(convw)[:, 0, :].astype(f32).copy(),
        convb=np.asarray(convb).reshape(128, 1).astype(f32),
        nconvb=(-np.asarray(convb).reshape(128, 1)).astype(f32),
        wxT=_wxt64(Wx),
        wdtT=np.asarray(Wdt).T.astype(f32).copy(),
        bdt=np.asarray(bdt).reshape(128, 1).astype(f32),
        acols=(-np.exp(np.asarray(Alog))).astype(f32),
        dskip=np.asarray(Dskip).reshape(128, 1).astype(f32),
        woutT=np.asarray(Wout).T.astype(f32).copy(),
        bout=np.asarray(bout).reshape(64, 1).astype(f32),
    )




def _make_runner(nc):
    """Persistent jitted SPMD runner (mirrors bass2jax.run_bass_via_pjrt but
    caches the jitted callable and device-resident weight shards across calls)."""
    import jax
    import jax.numpy as jnp
    from jax.sharding import Mesh, PartitionSpec
    from jax.experimental.shard_map import shard_map
    from concourse import bass2jax, mybir as _mb
    bass2jax.install_neuronx_cc_hook()

    n_cores = 8
    in_names, out_names, out_avals, zero_outs = [], [], [], []
    partition_name = nc.partition_id_tensor.name if nc.partition_id_tensor else None
    for alloc in nc.m.functions[0].allocations:
        if not isinstance(alloc, _mb.MemoryLocationSet):
            continue
        name = alloc.memorylocations[0].name
        if alloc.kind == "ExternalInput":
            if name != partition_name:
                in_names.append(name)
        elif alloc.kind == "ExternalOutput":
            shape = tuple(alloc.tensor_shape)
            dtype = _mb.dt.np(alloc.dtype)
            out_names.append(name)
            out_avals.append(jax.core.ShapedArray(shape, dtype))
            zero_outs.append(np.zeros(shape, dtype))
    n_params = len(in_names)
    all_names = list(in_names) + list(out_names)
    if partition_name is not None:
        all_names.append(partition_name)

    def _body(*args):
        operands = list(args)
        if partition_name is not None:
            operands.append(bass2jax.partition_id_tensor())
        outs = bass2jax._bass_exec_p.bind(
            *operands, out_avals=tuple(out_avals), in_names=tuple(all_names),
            out_names=tuple(out_names), lowering_input_output_aliases=(),
            sim_require_finite=True, sim_require_nnan=True, nc=nc)
        return tuple(outs)

    devices = jax.devices()[:n_cores]
    mesh = Mesh(np.asarray(devices), ("core",))
    nin = n_params + len(out_names)
    sharded = jax.jit(shard_map(
        _body, mesh=mesh, in_specs=(PartitionSpec("core"),) * nin,
        out_specs=(PartitionSpec("core"),) * len(out_names), check_rep=False))

    def run(in_maps):
        concat_in = [np.concatenate([np.asarray(in_maps[c][nm])
                                     for c in range(n_cores)], axis=0)
                     for nm in in_names]
        concat_zeros = [np.zeros((n_cores * z.shape[0], *z.shape[1:]), z.dtype)
                        for z in zero_outs]
        out_arrs = sharded(*concat_in, *concat_zeros)
        out_arrs = [np.asarray(a) for a in out_arrs]
        return [{nm: out_arrs[i].reshape(n_cores, *out_avals[i].shape)[c]
                 for i, nm in enumerate(out_names)} for c in range(n_cores)]

    return run


def get_nc():
    if "nc" not in _CACHE:
        _CACHE["nc"] = _build_nc()
    return _CACHE["nc"]


def kernel(x, x2, **kw):
    nc = get_nc()
    wts = _prep_weights(**kw)
    xf = np.asarray(x, np.float32).reshape(8, CH, L)
    x2f = np.asarray(x2, np.float32).reshape(8, CH, L)
    in_maps = []
    for b in range(8):
        m = dict(wts)
        m["xb"] = np.ascontiguousarray(xf[b])
        m["x2b"] = np.ascontiguousarray(x2f[b])
        in_maps.append(m)
    if "runner" not in _CACHE:
        try:
            _CACHE["runner"] = _make_runner(nc)
        except Exception:
            _CACHE["runner"] = None
    if _CACHE["runner"] is not None:
        results = _CACHE["runner"](in_maps)
    else:
        results = run_bass_kernel_spmd(nc, in_maps, core_ids=list(range(8))).results
    out = np.stack([results[b]["out"] for b in range(8)], axis=0)
    return out.reshape(8, CH, H, W).astype(np.float32)


# revision 16
# speedup vs baseline: 42.2011x; 33.1336x over previous
"""Trainium2 Bass kernel for nn_Directionalmamba (B=8, CH=256, H=W=64).

Sharding: data-parallel over batch — each of the 8 NeuronCores runs one batch
element end-to-end (1x1 conv + BN/ReLU front-end, 4 directional selective
scans, 4 directional 5-tap conv branches, output assembly). No collectives.

Key mappings per core:
  - 1x1 conv / all projections: fp32r matmuls (F=512 chunks, PSUM accum).
  - Directional orders produced via layout tricks: transposed eviction for
    dir1; row-doubled layout for the two diagonal dirs so the (i+j)%64 /
    (j-i)%64 gathers become affine access patterns.
  - Conv branches = 5 shifted-AP matmuls on a zero-padded SBUF tile.
  - Selective scan: hardware prefix scan (tensor_tensor_scan) per (n, chunk)
    on GpSimd; a=exp(dt*A[:,n]) on ACT via per-partition scale; b=u*B with
    B/C broadcast through DMA-replicated (t,n)-interleaved bf16 rows;
    y=sum_n h*C via a masked cumulative scan (segment sum).
"""
import numpy as np

import concourse.bass as bass
import concourse.tile as tile
from concourse import mybir, bacc
from concourse.bass_utils import run_bass_kernel_spmd

F32 = mybir.dt.float32
F32R = mybir.dt.float32r
BF16 = mybir.dt.float16
AOT = mybir.AluOpType
ACTF = mybir.ActivationFunctionType

CH = 256
D, E, N, DTR, KCONV = 64, 128, 16, 4, 4
H = W = 64
L = H * W              # 4096
FC = 512               # matmul moving-dim chunk
NFC = L // FC          # 8
TC = 256               # scan time chunk
NTC = L // TC          # 16
PADW = 68
EPS = 1e-5

_CACHE = {}


def _m_ap(v, dims, extra_offset=0, keep_partition=True):
    """Manual access pattern: replace free dims of AP `v` with `dims`
    ([step, count] pairs, arbitrary steps) at `extra_offset` elements."""
    w = v.copy()
    w.offset = v.offset + extra_offset
    lead = [list(v.ap[0])] if keep_partition else []
    w.ap = mybir.VecI64Pair(lead + [list(d) for d in dims])
    return w


def _build_nc():
    nc = bacc.Bacc("TRN2", target_bir_lowering=False, debug=False)
    ap = {}

    def din(name, shape, dt=F32):
        ap[name] = nc.dram_tensor(name, list(shape), dt, kind="ExternalInput").ap()

    din("xb", (CH, L), F32R)
    din("x2b", (CH, L), F32R)
    din("w1t", (4, 128, 2, 128), F32R)   # [kk][k][m][j]: lhsT for 1x1 (BN-folded)
    din("b1f", (128, 2))
    din("wbr", (2, 128, 20, 64), F32R)   # [kk][k][dir*5+tap][o]
    din("bbr", (64, 4))
    din("winT", (64, 256), F32R)
    din("bin2", (128, 2))
    din("convw", (128, 4))
    din("convb", (128, 1))
    din("nconvb", (128, 1))
    din("wxT", (128, 64), F32R)
    din("wdtT", (4, 128), F32R)
    din("bdt", (128, 1))
    din("acols", (128, 16))
    din("dskip", (128, 1))
    din("woutT", (128, 64), F32R)
    din("bout", (64, 1))

    out_ap = nc.dram_tensor("out", [CH, L], F32, kind="ExternalOutput").ap()
    xc01 = nc.dram_tensor("xc01", [128, L], F32R).ap()
    xcdbl = nc.dram_tensor("xcdbl", [128, 2 * L], F32R).ap()
    cbr = nc.dram_tensor("cbr", [4, 64, L], F32).ap()
    bcp = nc.dram_tensor("bcp", [4, 2, 16 * L], BF16).ap()
    md = nc.dram_tensor("md", [4, 64, 2 * L], F32).ap()

    with tile.TileContext(nc) as tc:
        _body(tc, ap, out_ap, xc01, xcdbl, cbr, bcp, md)
    nc.compile()
    return nc


def _body(tc, ap, out_ap, xc01, xcdbl, cbr, bcp, md):
    nc = tc.nc
    with nc.allow_low_precision(reason="f32r tags are byte-identical to f32"), \
         tc.tile_pool(name="wts", bufs=1) as wpool:
        _body2(tc, wpool, ap, out_ap, xc01, xcdbl, cbr, bcp, md)


def _body2(tc, wpool, ap, out_ap, xc01, xcdbl, cbr, bcp, md):
    nc = tc.nc

    def wtile(name, shape, dt=F32):
        t = wpool.tile(list(shape), dt, name=name)
        nc.sync.dma_start(t[:], ap[name])
        return t

    winT = wtile("winT", (64, 256), F32R)
    bin2 = wtile("bin2", (128, 2))
    convw = wtile("convw", (128, 4))
    convb = wtile("convb", (128, 1))
    nconvb = wtile("nconvb", (128, 1))
    wxT = wtile("wxT", (128, 64), F32R)
    wdtT = wtile("wdtT", (4, 128), F32R)
    bdt = wtile("bdt", (128, 1))
    acols = wtile("acols", (128, 16))
    dskip = wtile("dskip", (128, 1))
    woutT = wtile("woutT", (128, 64), F32R)
    bout = wtile("bout", (64, 1))
    b1f = wtile("b1f", (128, 2))
    bbr = wtile("bbr", (64, 4))

    # ================= PHASE A: 1x1 conv + BN/ReLU + branches =================
    with tc.tile_pool(name="phA", bufs=1) as pa, \
         tc.tile_pool(name="phAp", bufs=3, space="PSUM") as pap:
        w1t = pa.tile([128, 4, 2, 128], F32R)
        nc.sync.dma_start(w1t[:], ap["w1t"].rearrange("a k b m -> k a b m"))
        wbr = pa.tile([128, 2, 20, 64], F32R)
        nc.sync.dma_start(wbr[:], ap["wbr"].rearrange("a k c m -> k a c m"))

        xk = []
        for i, (src, half) in enumerate([("xb", 0), ("xb", 1), ("x2b", 0), ("x2b", 1)]):
            t = pa.tile([128, L], F32R, tag=f"xk{i}", name=f"xk{i}")
            nc.sync.dma_start(t[:], ap[src][128 * half:128 * (half + 1), :])
            xk.append(t)

        pads = [pa.tile([128, PADW * PADW], F32R, tag=f"pad{i}", name=f"pad{i}")
                for i in range(2)]
        nc.vector.memset(pads[0][:].bitcast(F32), 0.0)
        nc.vector.memset(pads[1][:].bitcast(F32), 0.0)
        xc01_sb = pa.tile([128, L], F32R)
        xcdbl_sb = pa.tile([128, 2 * L], F32R)

        for m in range(2):
            for fc in range(NFC):
                ps = pap.tile([128, FC], F32, tag="ps1x1")
                for kk in range(4):
                    nc.tensor.matmul(
                        ps[:], w1t[:, kk, m, :].bitcast(F32R),
                        xk[kk][:, fc * FC:(fc + 1) * FC].bitcast(F32R),
                        start=(kk == 0), stop=(kk == 3))
                ps3 = ps[:].rearrange("p (i j) -> p i j", i=8, j=64)
                padv = pads[m][:].rearrange("p (r c) -> p r c", r=PADW, c=PADW)
                nc.scalar.activation(
                    padv[:, 2 + 8 * fc:2 + 8 * fc + 8, 2:66], ps3,
                    ACTF.Relu, bias=b1f[:, m:m + 1])
                if m == 0:
                    nc.scalar.activation(
                        xc01_sb[0:64, fc * FC:(fc + 1) * FC], ps[0:64, :],
                        ACTF.Relu, bias=b1f[0:64, 0:1])
                    tr = xc01_sb[64:128, :].rearrange(
                        "p (j i) -> p i j", j=64, i=64)[:, 8 * fc:8 * fc + 8, :]
                    nc.scalar.activation(tr, ps3[64:128], ACTF.Relu,
                                         bias=b1f[64:128, 0:1])
                else:
                    dblv = xcdbl_sb[:].rearrange("p (r c) -> p r c", r=64, c=128)
                    nc.scalar.activation(
                        dblv[:, 8 * fc:8 * fc + 8, 0:64], ps3,
                        ACTF.Relu, bias=b1f[:, 1:2])
                    nc.scalar.activation(
                        dblv[:, 8 * fc:8 * fc + 8, 64:128], ps3,
                        ACTF.Relu, bias=b1f[:, 1:2])
        nc.sync.dma_start(xc01, xc01_sb[:])
        nc.sync.dma_start(xcdbl, xcdbl_sb[:])

        # branches: taps (dr, dc): c1 (0,s) c2 (s,0) c3 (s,-s) c4 (s,-s)
        tap_dirs = [(0, 1), (1, 0), (1, -1), (1, -1)]
        cbr_sb = pa.tile([64, L], F32)
        for d in range(4):
            sr, sc = tap_dirs[d]
            for fc in range(NFC):
                psb = pap.tile([64, FC], F32, tag="psbr")
                first = True
                for s in range(-2, 3):
                    dr, dc = sr * s, sc * s
                    for kk in range(2):
                        rhs = pads[kk][:].rearrange(
                            "p (r c) -> p r c", r=PADW, c=PADW)[
                            :, 2 + 8 * fc + dr:2 + 8 * fc + dr + 8,
                            2 + dc:2 + dc + 64]
                        nc.tensor.matmul(
                            psb[:], wbr[:, kk, d * 5 + s + 2, :].bitcast(F32R),
                            rhs.bitcast(F32R),
                            start=first, stop=(s == 2 and kk == 1))
                        first = False
                nc.scalar.activation(
                    cbr_sb[:, fc * FC:(fc + 1) * FC], psb[:],
                    ACTF.Identity, bias=bbr[:, d:d + 1])
            nc.sync.dma_start(cbr[d], cbr_sb[:])

    # ================= PHASE B: 4 directional mamba sequences =================
    with tc.tile_pool(name="phB", bufs=1) as pb, \
         tc.tile_pool(name="phBr", bufs=2) as pbr, \
         tc.tile_pool(name="phBs", bufs=2) as pbs, \
         tc.tile_pool(name="phBp", bufs=4, space="PSUM") as pbp:
        for d in range(4):
            xi_pad = pb.tile([128, L + 32], F32R, tag="xi_pad")
            zs = pb.tile([128, L], F32, tag="zs")
            dt = pb.tile([128, L], F32, tag="dt")
            u = pb.tile([128, L], F32R, tag="u")
            dbl_sb = pb.tile([128, L], F32R, tag="dbl")
            bc16 = pb.tile([64, L], BF16, tag="bc16")
            b_all = pb.tile([128, 16 * TC], F32, tag="b_o_shared")

            nc.vector.memset(xi_pad[:, 0:3].bitcast(F32), 0.0)

            # ---- in-proj ----
            for fc in range(NFC):
                rr = pbr.tile([64, FC], F32R, tag="rhs")
                if d == 0:
                    nc.sync.dma_start(rr[:], xc01[0:64, fc * FC:(fc + 1) * FC])
                elif d == 1:
                    nc.sync.dma_start(rr[:], xc01[64:128, fc * FC:(fc + 1) * FC])
                elif d == 2:
                    src = _m_ap(xcdbl[0:64, :], [[129, 8], [1, 64]], 129 * 8 * fc)
                    nc.sync.dma_start(rr[:], src)
                else:
                    src = _m_ap(xcdbl[64:128, :], [[127, 8], [1, 64]],
                                64 + 127 * 8 * fc)
                    nc.sync.dma_start(rr[:], src)
                pxi = pbp.tile([128, FC], F32, tag="psB", name="pxi")
                nc.tensor.matmul(pxi[:], winT[:, 0:128].bitcast(F32R),
                                 rr[:].bitcast(F32R), start=True, stop=True)
                nc.scalar.activation(xi_pad[:, 3 + fc * FC:3 + (fc + 1) * FC],
                                     pxi[:], ACTF.Identity, bias=bin2[:, 0:1])
                pz = pbp.tile([128, FC], F32, tag="psB", name="pz")
                nc.tensor.matmul(pz[:], winT[:, 128:256].bitcast(F32R),
                                 rr[:].bitcast(F32R), start=True, stop=True)
                nc.scalar.activation(zs[:, fc * FC:(fc + 1) * FC], pz[:],
                                     ACTF.Identity, bias=bin2[:, 1:2])

            # ---- causal depthwise conv1d + SiLU -> u holds xic ----
            nc.vector.tensor_scalar(u[:], xi_pad[:, 0:L], convw[:, 0:1],
                                    None, AOT.mult)
            for k in range(1, 4):
                nc.vector.scalar_tensor_tensor(
                    u[:], xi_pad[:, k:k + L], convw[:, k:k + 1], u[:],
                    AOT.mult, AOT.add)
            # silu(u + convb) = (u+convb) * 1/(1+exp(-(u+convb)))
            nc.scalar.activation(dt[:], u[:], ACTF.Exp, scale=-1.0,
                                 bias=nconvb[:, 0:1])
            nc.vector.tensor_scalar(dt[:], dt[:], 1.0, None, AOT.add)
            nc.vector.reciprocal(dt[:], dt[:])
            nc.vector.scalar_tensor_tensor(u[:], u[:], convb[:, 0:1], dt[:],
                                           AOT.add, AOT.mult)

            # ---- dbl projection (dt_raw | B | C), dt projection ----
            for fc in range(NFC):
                pdb = pbp.tile([64, FC], F32, tag="psB", name="pdb")
                nc.tensor.matmul(pdb[:], wxT[:].bitcast(F32R),
                                 u[:, fc * FC:(fc + 1) * FC].bitcast(F32R),
                                 start=True, stop=True)
                nc.scalar.activation(dbl_sb[0:64, fc * FC:(fc + 1) * FC],
                                     pdb[:], ACTF.Copy)
            for fc in range(NFC):
                pdt = pbp.tile([128, FC], F32, tag="psB", name="pdt")
                nc.tensor.matmul(pdt[:], wdtT[:].bitcast(F32R),
                                 dbl_sb[0:4, fc * FC:(fc + 1) * FC].bitcast(F32R),
                                 start=True, stop=True)
                nc.scalar.activation(dt[:, fc * FC:(fc + 1) * FC], pdt[:],
                                     ACTF.Exp, bias=bdt[:, 0:1])
                nc.scalar.activation(dt[:, fc * FC:(fc + 1) * FC],
                                     dt[:, fc * FC:(fc + 1) * FC],
                                     ACTF.Ln, bias=1.0)

            # g = (xic * Dskip) * zs stored into xi_pad[:, 0:L] (xi_pad dead)
            nc.vector.scalar_tensor_tensor(xi_pad[:, 0:L], u[:], dskip[:, 0:1],
                                           zs[:], AOT.mult, AOT.mult)
            # u := dt * xic (in place; after dbl used xic)
            nc.vector.tensor_tensor(u[:], u[:], dt[:], AOT.mult)

            # export B,C as bf16 (t,n)-interleaved rows to DRAM
            nc.scalar.activation(bc16[32:64, :], dbl_sb[32:64, :], ACTF.Copy)
            for bi in range(2):
                dst = _m_ap(bcp[d, bi, :], [[1, 16], [16, L]],
                            keep_partition=False)
                nc.sync.dma_start(dst, bc16[32 + 16 * bi:48 + 16 * bi, :])

            # zs := silu(zs) using exp/recip (dbl_sb rows free as scratch)
            nc.scalar.activation(dbl_sb[:], zs[:], ACTF.Exp, scale=-1.0)
            nc.vector.tensor_scalar(dbl_sb[:], dbl_sb[:], 1.0, None, AOT.add)
            nc.vector.reciprocal(dbl_sb[:], dbl_sb[:])
            nc.gpsimd.tensor_tensor(zs[:], zs[:], dbl_sb[:], AOT.mult)

            # ---- scan chunks (software-pipelined y-stage) ----
            h_prev = None
            c_prev = None
            for c in range(NTC + 1):
                if c < NTC:
                    brep = pbs.tile([128, 16 * TC], BF16, tag="brep")
                    nc.sync.dma_start(
                        brep[:], bcp[d, 0, 16 * TC * c:16 * TC * (c + 1)]
                        .partition_broadcast(128))
                    crep = pbs.tile([128, 16 * TC], BF16, tag="crep")
                    nc.sync.dma_start(
                        crep[:], bcp[d, 1, 16 * TC * c:16 * TC * (c + 1)]
                        .partition_broadcast(128))
                    u_bc = _m_ap(u[:], [[1, TC], [0, 16]], TC * c)
                    nc.gpsimd.tensor_tensor(
                        b_all[:].rearrange("p (t n) -> p t n", n=16),
                        u_bc, brep[:].rearrange("p (t n) -> p t n", n=16),
                        AOT.mult)
                    h_all = pbs.tile([128, 16 * TC], F32, tag="h_all")
                    hv = h_all[:].rearrange("p (t n) -> p t n", n=16)
                    bv = b_all[:].rearrange("p (t n) -> p t n", n=16)
                    for n in range(16):
                        a_n = pbs.tile([128, TC], F32, tag="a_n")
                        nc.scalar.activation(a_n[:], dt[:, TC * c:TC * (c + 1)],
                                             ACTF.Exp, scale=acols[:, n:n + 1])
                        if c == 0:
                            init = 0.0
                        else:
                            init = h_prev[:].rearrange(
                                "p (t n) -> p t n", n=16)[:, TC - 1:TC, n]
                        nc.vector.tensor_tensor_scan(
                            hv[:, :, n], a_n[:], bv[:, :, n], init,
                            AOT.mult, AOT.add)
                if c > 0:
                    cm1 = c - 1
                    eng = nc.vector if (cm1 % 2 == 0) else nc.gpsimd
                    eng.tensor_tensor(h_prev[:], h_prev[:], c_prev[:], AOT.mult)
                    pv = h_prev[:].rearrange("p (t n) -> p t n", n=16)
                    for half in (8, 4, 2, 1):
                        nc.gpsimd.tensor_tensor(pv[:, :, 0:half],
                                                pv[:, :, 0:half],
                                                pv[:, :, half:2 * half], AOT.add)
                    tmp = pbs.tile([128, TC], F32, tag="tmp")
                    nc.vector.tensor_tensor(tmp[:], pv[:, :, 0],
                                            zs[:, TC * cm1:TC * c], AOT.mult)
                    nc.gpsimd.tensor_tensor(
                        xi_pad[:, TC * cm1:TC * c], tmp[:],
                        xi_pad[:, TC * cm1:TC * c], AOT.add)
                if c < NTC:
                    h_prev = h_all
                    c_prev = crep

            # ---- out-proj ----
            o_sb = pb.tile([64, L], F32, tag="b_o_shared")
            for fc in range(NFC):
                po = pbp.tile([64, FC], F32, tag="psB", name="po")
                nc.tensor.matmul(po[:], woutT[:].bitcast(F32R),
                                 xi_pad[:, fc * FC:(fc + 1) * FC].bitcast(F32R),
                                 start=True, stop=True)
                nc.scalar.activation(o_sb[:, fc * FC:(fc + 1) * FC], po[:],
                                     ACTF.Identity, bias=bout[:, 0:1])
            nc.sync.dma_start(md[d][:, 0:L], o_sb[:])
            if d >= 2:
                nc.sync.dma_start(md[d][:, L:2 * L], o_sb[:])

    # ================= PHASE C: assembly =================
    with tc.tile_pool(name="phC", bufs=2) as pc:
        for d in range(4):
            mo = pc.tile([64, 2 * L], F32, tag="mo")
            if d < 2:
                nc.sync.dma_start(mo[:, 0:L], md[d][:, 0:L])
            else:
                nc.sync.dma_start(mo[:], md[d])
            cb = pc.tile([64, L], F32, tag="cb")
            nc.sync.dma_start(cb[:], cbr[d])
            ofin = pc.tile([64, L], F32, tag="ofin")
            if d == 0:
                src = mo[:, 0:L]
            elif d == 1:
                src = _m_ap(mo[:], [[1, 64], [64, 64]])
            elif d == 2:
                src = _m_ap(mo[:], [[-63, 64], [64, 64]], L)
            else:
                src = _m_ap(mo[:], [[65, 64], [64, 64]])
            nc.vector.tensor_tensor(ofin[:], src, cb[:], AOT.add)
            nc.sync.dma_start(out_ap[64 * d:64 * (d + 1), :], ofin[:])




def _wxt64(Wx):
    wt = np.asarray(Wx).T.astype(np.float32)  # (128, 36)
    out = np.zeros((128, 64), np.float32)
    out[:, 0:4] = wt[:, 0:4]
    out[:, 32:48] = wt[:, 4:20]
    out[:, 48:64] = wt[:, 20:36]
    return out

def _prep_weights(w1, b1, bn_g, bn_b, bn_m, bn_v,
                  hconv_w, hconv_b, wconv_w, wconv_b, d19_w, d19_b, d37_w,
                  d37_b, Win, bin_, convw, convb, Wx, Wdt, bdt, Alog, Dskip,
                  Wout, bout):
    f32 = np.float32
    scale = (bn_g / np.sqrt(bn_v + EPS)).astype(f32)
    w1f = (np.asarray(w1)[:, :, 0, 0] * scale[:, None]).astype(f32)  # (256, 512)
    b1fv = ((np.asarray(b1) - bn_m) * scale + bn_b).astype(f32)

    w1t = np.zeros((4, 128, 2, 128), f32)
    for kk in range(4):
        for m in range(2):
            w1t[kk, :, m, :] = w1f[m * 128:(m + 1) * 128,
                                   kk * 128:(kk + 1) * 128].T
    b1f = np.stack([b1fv[0:128], b1fv[128:256]], axis=1)

    # branch taps: weight[s] for offset pattern (see _body tap_dirs)
    taps = np.zeros((4, 5, 64, 256), f32)
    for s in range(-2, 3):
        taps[0, s + 2] = np.asarray(hconv_w)[:, :, 0, s + 2]
        taps[1, s + 2] = np.asarray(wconv_w)[:, :, s + 2, 0]
        taps[2, s + 2] = np.asarray(d19_w)[:, :, s + 2, 0]
        taps[3, s + 2] = np.asarray(d37_w)[:, :, 0, 2 - s]
    wbr = np.zeros((2, 128, 20, 64), f32)
    for kk in range(2):
        for idx in range(20):
            dd, ss = idx // 5, idx % 5
            wbr[kk, :, idx, :] = taps[dd, ss, :, kk * 128:(kk + 1) * 128].T
    bbr = np.stack([hconv_b, wconv_b, d19_b, d37_b], axis=1).astype(f32)

    return dict(
        w1t=w1t, b1f=b1f, wbr=wbr, bbr=bbr,
        winT=np.asarray(Win).T.astype(f32).copy(),
        bin2=np.stack([bin_[0:128], bin_[128:256]], axis=1).astype(f32),
        convw=np.asarray(convw)[:, 0, :].astype(f32).copy(),
        convb=np.asarray(convb).reshape(128, 1).astype(f32),
        nconvb=(-np.asarray(convb).reshape(128, 1)).astype(f32),
        wxT=_wxt64(Wx),
        wdtT=np.asarray(Wdt).T.astype(f32).copy(),
        bdt=np.asarray(bdt).reshape(128, 1).astype(f32),
        acols=(-np.exp(np.asarray(Alog))).astype(f32),
        dskip=np.asarray(Dskip).reshape(128, 1).astype(f32),
        woutT=np.asarray(Wout).T.astype(f32).copy(),
        bout=np.asarray(bout).reshape(64, 1).astype(f32),
    )




def _make_runner(nc):
    """Persistent jitted SPMD runner (mirrors bass2jax.run_bass_via_pjrt but
    caches the jitted callable and device-resident weight shards across calls)."""
    import jax
    import jax.numpy as jnp
    from jax.sharding import Mesh, PartitionSpec
    from jax.experimental.shard_map import shard_map
    from concourse import bass2jax, mybir as _mb
    bass2jax.install_neuronx_cc_hook()

    n_cores = 8
    in_names, out_names, out_avals, zero_outs = [], [], [], []
    partition_name = nc.partition_id_tensor.name if nc.partition_id_tensor else None
    for alloc in nc.m.functions[0].allocations:
        if not isinstance(alloc, _mb.MemoryLocationSet):
            continue
        name = alloc.memorylocations[0].name
        if alloc.kind == "ExternalInput":
            if name != partition_name:
                in_names.append(name)
        elif alloc.kind == "ExternalOutput":
            shape = tuple(alloc.tensor_shape)
            dtype = _mb.dt.np(alloc.dtype)
            out_names.append(name)
            out_avals.append(jax.core.ShapedArray(shape, dtype))
            zero_outs.append(np.zeros(shape, dtype))
    n_params = len(in_names)
    all_names = list(in_names) + list(out_names)
    if partition_name is not None:
        all_names.append(partition_name)

    def _body(*args):
        operands = list(args)
        if partition_name is not None:
            operands.append(bass2jax.partition_id_tensor())
        outs = bass2jax._bass_exec_p.bind(
            *operands, out_avals=tuple(out_avals), in_names=tuple(all_names),
            out_names=tuple(out_names), lowering_input_output_aliases=(),
            sim_require_finite=True, sim_require_nnan=True, nc=nc)
        return tuple(outs)

    devices = jax.devices()[:n_cores]
    mesh = Mesh(np.asarray(devices), ("core",))
    nin = n_params + len(out_names)
    sharded = jax.jit(shard_map(
        _body, mesh=mesh, in_specs=(PartitionSpec("core"),) * nin,
        out_specs=(PartitionSpec("core"),) * len(out_names), check_rep=False))

    _CACHE["sharded_fn"] = sharded

    def run(in_maps):
        concat_in = [np.concatenate([np.asarray(in_maps[c][nm])
                                     for c in range(n_cores)], axis=0)
                     for nm in in_names]
        concat_zeros = [np.zeros((n_cores * z.shape[0], *z.shape[1:]), z.dtype)
                        for z in zero_outs]
        out_arrs = sharded(*concat_in, *concat_zeros)
        out_arrs = [np.asarray(a) for a in out_arrs]
        return [{nm: out_arrs[i].reshape(n_cores, *out_avals[i].shape)[c]
                 for i, nm in enumerate(out_names)} for c in range(n_cores)]

    return run


def get_nc():
    if "nc" not in _CACHE:
        _CACHE["nc"] = _build_nc()
    return _CACHE["nc"]


def kernel(x, x2, **kw):
    nc = get_nc()
    wts = _prep_weights(**kw)
    xf = np.asarray(x, np.float32).reshape(8, CH, L)
    x2f = np.asarray(x2, np.float32).reshape(8, CH, L)
    in_maps = []
    for b in range(8):
        m = dict(wts)
        m["xb"] = np.ascontiguousarray(xf[b])
        m["x2b"] = np.ascontiguousarray(x2f[b])
        in_maps.append(m)
    if "runner" not in _CACHE:
        try:
            _CACHE["runner"] = _make_runner(nc)
        except Exception:
            _CACHE["runner"] = None
    if _CACHE["runner"] is not None:
        results = _CACHE["runner"](in_maps)
    else:
        results = run_bass_kernel_spmd(nc, in_maps, core_ids=list(range(8))).results
    out = np.stack([results[b]["out"] for b in range(8)], axis=0)
    return out.reshape(8, CH, H, W).astype(np.float32)


# revision 22
# speedup vs baseline: 44.0508x; 1.0438x over previous
"""Trainium2 Bass kernel for nn_Directionalmamba (B=8, CH=256, H=W=64).

Sharding: data-parallel over batch — each of the 8 NeuronCores runs one batch
element end-to-end (1x1 conv + BN/ReLU front-end, 4 directional selective
scans, 4 directional 5-tap conv branches, output assembly). No collectives.

Key mappings per core:
  - 1x1 conv / all projections: fp32r matmuls (F=512 chunks, PSUM accum).
  - Directional orders produced via layout tricks: transposed eviction for
    dir1; row-doubled layout for the two diagonal dirs so the (i+j)%64 /
    (j-i)%64 gathers become affine access patterns.
  - Conv branches = 5 shifted-AP matmuls on a zero-padded SBUF tile.
  - Selective scan: hardware prefix scan (tensor_tensor_scan) per (n, chunk)
    on GpSimd; a=exp(dt*A[:,n]) on ACT via per-partition scale; b=u*B with
    B/C broadcast through DMA-replicated (t,n)-interleaved bf16 rows;
    y=sum_n h*C via a masked cumulative scan (segment sum).
"""
import numpy as np

import concourse.bass as bass
import concourse.tile as tile
from concourse import mybir, bacc
from concourse.bass_utils import run_bass_kernel_spmd

F32 = mybir.dt.float32
F32R = mybir.dt.float32r
BF16 = mybir.dt.float16
AOT = mybir.AluOpType
ACTF = mybir.ActivationFunctionType

CH = 256
D, E, N, DTR, KCONV = 64, 128, 16, 4, 4
H = W = 64
L = H * W              # 4096
FC = 512               # matmul moving-dim chunk
NFC = L // FC          # 8
TC = 256               # scan time chunk
NTC = L // TC          # 16
PADW = 68
EPS = 1e-5

_CACHE = {}


def _m_ap(v, dims, extra_offset=0, keep_partition=True):
    """Manual access pattern: replace free dims of AP `v` with `dims`
    ([step, count] pairs, arbitrary steps) at `extra_offset` elements."""
    w = v.copy()
    w.offset = v.offset + extra_offset
    lead = [list(v.ap[0])] if keep_partition else []
    w.ap = mybir.VecI64Pair(lead + [list(d) for d in dims])
    return w


def _build_nc():
    nc = bacc.Bacc("TRN2", target_bir_lowering=False, debug=False)
    ap = {}

    def din(name, shape, dt=F32):
        ap[name] = nc.dram_tensor(name, list(shape), dt, kind="ExternalInput").ap()

    din("xb", (CH, L), F32R)
    din("x2b", (CH, L), F32R)
    din("w1t", (4, 128, 2, 128), F32R)   # [kk][k][m][j]: lhsT for 1x1 (BN-folded)
    din("b1f", (128, 2))
    din("wbr", (2, 128, 20, 64), F32R)   # [kk][k][dir*5+tap][o]
    din("bbr", (64, 4))
    din("winT", (64, 256), F32R)
    din("bin2", (128, 2))
    din("convw", (128, 4))
    din("convb", (128, 1))
    din("nconvb", (128, 1))
    din("wxT", (128, 64), F32R)
    din("wdtT", (4, 128), F32R)
    din("bdt", (128, 1))
    din("acols", (128, 16))
    din("dskip", (128, 1))
    din("woutT", (128, 64), F32R)
    din("bout", (64, 1))

    out_ap = nc.dram_tensor("out", [CH, L], F32, kind="ExternalOutput").ap()
    xc01 = nc.dram_tensor("xc01", [128, L], F32R).ap()
    xcdbl = nc.dram_tensor("xcdbl", [128, 2 * L], F32R).ap()
    cbr = nc.dram_tensor("cbr", [4, 64, L], F32).ap()
    bcp = nc.dram_tensor("bcp", [4, 2, 16 * L], BF16).ap()
    md = nc.dram_tensor("md", [4, 64, 2 * L], F32).ap()

    with tile.TileContext(nc) as tc:
        _body(tc, ap, out_ap, xc01, xcdbl, cbr, bcp, md)
    nc.compile()
    return nc


def _body(tc, ap, out_ap, xc01, xcdbl, cbr, bcp, md):
    nc = tc.nc
    with nc.allow_low_precision(reason="f32r tags are byte-identical to f32"), \
         tc.tile_pool(name="wts", bufs=1) as wpool:
        _body2(tc, wpool, ap, out_ap, xc01, xcdbl, cbr, bcp, md)


def _body2(tc, wpool, ap, out_ap, xc01, xcdbl, cbr, bcp, md):
    nc = tc.nc

    def wtile(name, shape, dt=F32):
        t = wpool.tile(list(shape), dt, name=name)
        nc.sync.dma_start(t[:], ap[name])
        return t

    winT = wtile("winT", (64, 256), F32R)
    bin2 = wtile("bin2", (128, 2))
    convw = wtile("convw", (128, 4))
    convb = wtile("convb", (128, 1))
    nconvb = wtile("nconvb", (128, 1))
    wxT = wtile("wxT", (128, 64), F32R)
    wdtT = wtile("wdtT", (4, 128), F32R)
    bdt = wtile("bdt", (128, 1))
    acols = wtile("acols", (128, 16))
    dskip = wtile("dskip", (128, 1))
    woutT = wtile("woutT", (128, 64), F32R)
    bout = wtile("bout", (64, 1))
    b1f = wtile("b1f", (128, 2))
    bbr = wtile("bbr", (64, 4))

    # ================= PHASE A: 1x1 conv + BN/ReLU + branches =================
    with tc.tile_pool(name="phA", bufs=1) as pa, \
         tc.tile_pool(name="phAp", bufs=3, space="PSUM") as pap:
        w1t = pa.tile([128, 4, 2, 128], F32R)
        nc.sync.dma_start(w1t[:], ap["w1t"].rearrange("a k b m -> k a b m"))
        wbr = pa.tile([128, 2, 20, 64], F32R)
        nc.sync.dma_start(wbr[:], ap["wbr"].rearrange("a k c m -> k a c m"))

        xk = []
        for i, (src, half) in enumerate([("xb", 0), ("xb", 1), ("x2b", 0), ("x2b", 1)]):
            t = pa.tile([128, L], F32R, tag=f"xk{i}", name=f"xk{i}")
            nc.sync.dma_start(t[:], ap[src][128 * half:128 * (half + 1), :])
            xk.append(t)

        pads = [pa.tile([128, PADW * PADW], F32R, tag=f"pad{i}", name=f"pad{i}")
                for i in range(2)]
        nc.vector.memset(pads[0][:].bitcast(F32), 0.0)
        nc.vector.memset(pads[1][:].bitcast(F32), 0.0)
        xc01_sb = pa.tile([128, L], F32R)
        xcdbl_sb = pa.tile([128, 2 * L], F32R)

        for m in range(2):
            for fc in range(NFC):
                ps = pap.tile([128, FC], F32, tag="ps1x1")
                for kk in range(4):
                    nc.tensor.matmul(
                        ps[:], w1t[:, kk, m, :].bitcast(F32R),
                        xk[kk][:, fc * FC:(fc + 1) * FC].bitcast(F32R),
                        start=(kk == 0), stop=(kk == 3))
                ps3 = ps[:].rearrange("p (i j) -> p i j", i=8, j=64)
                padv = pads[m][:].rearrange("p (r c) -> p r c", r=PADW, c=PADW)
                nc.scalar.activation(
                    padv[:, 2 + 8 * fc:2 + 8 * fc + 8, 2:66], ps3,
                    ACTF.Relu, bias=b1f[:, m:m + 1])
                if m == 0:
                    nc.scalar.activation(
                        xc01_sb[0:64, fc * FC:(fc + 1) * FC], ps[0:64, :],
                        ACTF.Relu, bias=b1f[0:64, 0:1])
                    tr = xc01_sb[64:128, :].rearrange(
                        "p (j i) -> p i j", j=64, i=64)[:, 8 * fc:8 * fc + 8, :]
                    nc.scalar.activation(tr, ps3[64:128], ACTF.Relu,
                                         bias=b1f[64:128, 0:1])
                else:
                    dblv = xcdbl_sb[:].rearrange("p (r c) -> p r c", r=64, c=128)
                    nc.scalar.activation(
                        dblv[:, 8 * fc:8 * fc + 8, 0:64], ps3,
                        ACTF.Relu, bias=b1f[:, 1:2])
                    nc.scalar.activation(
                        dblv[:, 8 * fc:8 * fc + 8, 64:128], ps3,
                        ACTF.Relu, bias=b1f[:, 1:2])
        nc.sync.dma_start(xc01, xc01_sb[:])
        nc.sync.dma_start(xcdbl, xcdbl_sb[:])

        # branches: taps (dr, dc): c1 (0,s) c2 (s,0) c3 (s,-s) c4 (s,-s)
        tap_dirs = [(0, 1), (1, 0), (1, -1), (1, -1)]
        cbr_sb = pa.tile([64, L], F32)
        for d in range(4):
            sr, sc = tap_dirs[d]
            for fc in range(NFC):
                psb = pap.tile([64, FC], F32, tag="psbr")
                first = True
                for s in range(-2, 3):
                    dr, dc = sr * s, sc * s
                    for kk in range(2):
                        rhs = pads[kk][:].rearrange(
                            "p (r c) -> p r c", r=PADW, c=PADW)[
                            :, 2 + 8 * fc + dr:2 + 8 * fc + dr + 8,
                            2 + dc:2 + dc + 64]
                        nc.tensor.matmul(
                            psb[:], wbr[:, kk, d * 5 + s + 2, :].bitcast(F32R),
                            rhs.bitcast(F32R),
                            start=first, stop=(s == 2 and kk == 1))
                        first = False
                nc.scalar.activation(
                    cbr_sb[:, fc * FC:(fc + 1) * FC], psb[:],
                    ACTF.Identity, bias=bbr[:, d:d + 1])
            nc.sync.dma_start(cbr[d], cbr_sb[:])

    # ================= PHASE B: 4 directional mamba sequences =================
    with tc.tile_pool(name="phB", bufs=1) as pb, \
         tc.tile_pool(name="phBr", bufs=2) as pbr, \
         tc.tile_pool(name="phBs", bufs=2) as pbs, \
         tc.tile_pool(name="phBp", bufs=4, space="PSUM") as pbp:
        for d in range(4):
            xi_pad = pb.tile([128, L + 32], F32R, tag="xi_pad")
            zs = pb.tile([128, L], F32, tag="zs")
            dt = pb.tile([128, L], F32, tag="dt")
            u = pb.tile([128, L], F32R, tag="u")
            dbl_sb = pb.tile([128, L], F32R, tag="dbl")
            bc16 = pb.tile([64, L], BF16, tag="bc16")

            nc.vector.memset(xi_pad[:, 0:3].bitcast(F32), 0.0)

            # ---- in-proj ----
            for fc in range(NFC):
                rr = pbr.tile([64, FC], F32R, tag="rhs")
                if d == 0:
                    nc.sync.dma_start(rr[:], xc01[0:64, fc * FC:(fc + 1) * FC])
                elif d == 1:
                    nc.sync.dma_start(rr[:], xc01[64:128, fc * FC:(fc + 1) * FC])
                elif d == 2:
                    src = _m_ap(xcdbl[0:64, :], [[129, 8], [1, 64]], 129 * 8 * fc)
                    nc.sync.dma_start(rr[:], src)
                else:
                    src = _m_ap(xcdbl[64:128, :], [[127, 8], [1, 64]],
                                64 + 127 * 8 * fc)
                    nc.sync.dma_start(rr[:], src)
                pxi = pbp.tile([128, FC], F32, tag="psB", name="pxi")
                nc.tensor.matmul(pxi[:], winT[:, 0:128].bitcast(F32R),
                                 rr[:].bitcast(F32R), start=True, stop=True)
                nc.scalar.activation(xi_pad[:, 3 + fc * FC:3 + (fc + 1) * FC],
                                     pxi[:], ACTF.Identity, bias=bin2[:, 0:1])
                pz = pbp.tile([128, FC], F32, tag="psB", name="pz")
                nc.tensor.matmul(pz[:], winT[:, 128:256].bitcast(F32R),
                                 rr[:].bitcast(F32R), start=True, stop=True)
                nc.scalar.activation(zs[:, fc * FC:(fc + 1) * FC], pz[:],
                                     ACTF.Identity, bias=bin2[:, 1:2])

            # ---- causal depthwise conv1d + SiLU -> u holds xic ----
            nc.vector.tensor_scalar(u[:], xi_pad[:, 0:L], convw[:, 0:1],
                                    None, AOT.mult)
            for k in range(1, 4):
                nc.vector.scalar_tensor_tensor(
                    u[:], xi_pad[:, k:k + L], convw[:, k:k + 1], u[:],
                    AOT.mult, AOT.add)
            # silu(u + convb) = (u+convb) * 1/(1+exp(-(u+convb)))
            nc.scalar.activation(dt[:], u[:], ACTF.Exp, scale=-1.0,
                                 bias=nconvb[:, 0:1])
            nc.vector.tensor_scalar(dt[:], dt[:], 1.0, None, AOT.add)
            nc.vector.reciprocal(dt[:], dt[:])
            nc.vector.scalar_tensor_tensor(u[:], u[:], convb[:, 0:1], dt[:],
                                           AOT.add, AOT.mult)

            # ---- dbl projection (dt_raw | B | C), dt projection ----
            for fc in range(NFC):
                pdb = pbp.tile([64, FC], F32, tag="psB", name="pdb")
                nc.tensor.matmul(pdb[:], wxT[:].bitcast(F32R),
                                 u[:, fc * FC:(fc + 1) * FC].bitcast(F32R),
                                 start=True, stop=True)
                nc.scalar.activation(dbl_sb[0:64, fc * FC:(fc + 1) * FC],
                                     pdb[:], ACTF.Copy)
            for fc in range(NFC):
                pdt = pbp.tile([128, FC], F32, tag="psB", name="pdt")
                nc.tensor.matmul(pdt[:], wdtT[:].bitcast(F32R),
                                 dbl_sb[0:4, fc * FC:(fc + 1) * FC].bitcast(F32R),
                                 start=True, stop=True)
                nc.scalar.activation(dt[:, fc * FC:(fc + 1) * FC], pdt[:],
                                     ACTF.Exp, bias=bdt[:, 0:1])
                nc.scalar.activation(dt[:, fc * FC:(fc + 1) * FC],
                                     dt[:, fc * FC:(fc + 1) * FC],
                                     ACTF.Ln, bias=1.0)

            # g = (xic * Dskip) * zs stored into xi_pad[:, 0:L] (xi_pad dead)
            nc.vector.scalar_tensor_tensor(xi_pad[:, 0:L], u[:], dskip[:, 0:1],
                                           zs[:], AOT.mult, AOT.mult)
            # u := dt * xic (in place; after dbl used xic)
            nc.vector.tensor_tensor(u[:], u[:], dt[:], AOT.mult)

            # export B,C as bf16 (t,n)-interleaved rows to DRAM
            nc.scalar.activation(bc16[32:64, :], dbl_sb[32:64, :], ACTF.Copy)
            for bi in range(2):
                dst = _m_ap(bcp[d, bi, :], [[1, 16], [16, L]],
                            keep_partition=False)
                nc.sync.dma_start(dst, bc16[32 + 16 * bi:48 + 16 * bi, :])

            # zs := silu(zs) using exp/recip (dbl_sb rows free as scratch)
            nc.scalar.activation(dbl_sb[:], zs[:], ACTF.Exp, scale=-1.0)
            nc.vector.tensor_scalar(dbl_sb[:], dbl_sb[:], 1.0, None, AOT.add)
            nc.vector.reciprocal(dbl_sb[:], dbl_sb[:])
            nc.gpsimd.tensor_tensor(zs[:], zs[:], dbl_sb[:], AOT.mult)

            # ---- scan chunks (software-pipelined y-stage) ----
            h_prev = None
            c_prev = None
            for c in range(NTC + 1):
                if c < NTC:
                    b_all = pbs.tile([128, 16 * TC], F32, tag="b_all")
                    brep = pbs.tile([128, 16 * TC], BF16, tag="brep", bufs=1)
                    nc.sync.dma_start(
                        brep[:], bcp[d, 0, 16 * TC * c:16 * TC * (c + 1)]
                        .partition_broadcast(128))
                    crep = pbs.tile([128, 16 * TC], BF16, tag="crep")
                    nc.sync.dma_start(
                        crep[:], bcp[d, 1, 16 * TC * c:16 * TC * (c + 1)]
                        .partition_broadcast(128))
                    u_bc = _m_ap(u[:], [[1, TC], [0, 16]], TC * c)
                    nc.gpsimd.tensor_tensor(
                        b_all[:].rearrange("p (t n) -> p t n", n=16),
                        u_bc, brep[:].rearrange("p (t n) -> p t n", n=16),
                        AOT.mult)
                    h_all = pbs.tile([128, 16 * TC], F32, tag="h_all")
                    hv = h_all[:].rearrange("p (t n) -> p t n", n=16)
                    bv = b_all[:].rearrange("p (t n) -> p t n", n=16)
                    for n in range(16):
                        a_n = pbs.tile([128, TC], F32, tag="a_n")
                        nc.scalar.activation(a_n[:], dt[:, TC * c:TC * (c + 1)],
                                             ACTF.Exp, scale=acols[:, n:n + 1])
                        if c == 0:
                            init = 0.0
                        else:
                            init = h_prev[:].rearrange(
                                "p (t n) -> p t n", n=16)[:, TC - 1:TC, n]
                        nc.vector.tensor_tensor_scan(
                            hv[:, :, n], a_n[:], bv[:, :, n], init,
                            AOT.mult, AOT.add)
                if c > 0:
                    cm1 = c - 1
                    nc.vector.tensor_tensor(h_prev[:], h_prev[:],
                                            c_prev[:], AOT.mult)
                    pv = h_prev[:].rearrange("p (t n) -> p t n", n=16)
                    for half in (8, 4, 2, 1):
                        nc.gpsimd.tensor_tensor(pv[:, :, 0:half],
                                                pv[:, :, 0:half],
                                                pv[:, :, half:2 * half], AOT.add)
                    tmp = pbs.tile([128, TC], F32, tag="a_n")
                    nc.vector.tensor_tensor(tmp[:], pv[:, :, 0],
                                            zs[:, TC * cm1:TC * c], AOT.mult)
                    nc.gpsimd.tensor_tensor(
                        xi_pad[:, TC * cm1:TC * c], tmp[:],
                        xi_pad[:, TC * cm1:TC * c], AOT.add)
                if c < NTC:
                    h_prev = h_all
                    c_prev = crep

            # ---- out-proj ----
            o_sb = pb.tile([64, L], F32, tag="o_sb")
            for fc in range(NFC):
                po = pbp.tile([64, FC], F32, tag="psB", name="po")
                nc.tensor.matmul(po[:], woutT[:].bitcast(F32R),
                                 xi_pad[:, fc * FC:(fc + 1) * FC].bitcast(F32R),
                                 start=True, stop=True)
                nc.scalar.activation(o_sb[:, fc * FC:(fc + 1) * FC], po[:],
                                     ACTF.Identity, bias=bout[:, 0:1])
            nc.sync.dma_start(md[d][:, 0:L], o_sb[:])
            if d >= 2:
                nc.sync.dma_start(md[d][:, L:2 * L], o_sb[:])

    # ================= PHASE C: assembly =================
    with tc.tile_pool(name="phC", bufs=2) as pc:
        for d in range(4):
            mo = pc.tile([64, 2 * L], F32, tag="mo")
            if d < 2:
                nc.sync.dma_start(mo[:, 0:L], md[d][:, 0:L])
            else:
                nc.sync.dma_start(mo[:], md[d])
            cb = pc.tile([64, L], F32, tag="cb")
            nc.sync.dma_start(cb[:], cbr[d])
            ofin = pc.tile([64, L], F32, tag="ofin")
            if d == 0:
                src = mo[:, 0:L]
            elif d == 1:
                src = _m_ap(mo[:], [[1, 64], [64, 64]])
            elif d == 2:
                src = _m_ap(mo[:], [[-63, 64], [64, 64]], L)
            else:
                src = _m_ap(mo[:], [[65, 64], [64, 64]])
            nc.vector.tensor_tensor(ofin[:], src, cb[:], AOT.add)
            nc.sync.dma_start(out_ap[64 * d:64 * (d + 1), :], ofin[:])




def _wxt64(Wx):
    wt = np.asarray(Wx).T.astype(np.float32)  # (128, 36)
    out = np.zeros((128, 64), np.float32)
    out[:, 0:4] = wt[:, 0:4]
    out[:, 32:48] = wt[:, 4:20]
    out[:, 48:64] = wt[:, 20:36]
    return out

def _prep_weights(w1, b1, bn_g, bn_b, bn_m, bn_v,
                  hconv_w, hconv_b, wconv_w, wconv_b, d19_w, d19_b, d37_w,
                  d37_b, Win, bin_, convw, convb, Wx, Wdt, bdt, Alog, Dskip,
                  Wout, bout):
    f32 = np.float32
    scale = (bn_g / np.sqrt(bn_v + EPS)).astype(f32)
    w1f = (np.asarray(w1)[:, :, 0, 0] * scale[:, None]).astype(f32)  # (256, 512)
    b1fv = ((np.asarray(b1) - bn_m) * scale + bn_b).astype(f32)

    w1t = np.zeros((4, 128, 2, 128), f32)
    for kk in range(4):
        for m in range(2):
            w1t[kk, :, m, :] = w1f[m * 128:(m + 1) * 128,
                                   kk * 128:(kk + 1) * 128].T
    b1f = np.stack([b1fv[0:128], b1fv[128:256]], axis=1)

    # branch taps: weight[s] for offset pattern (see _body tap_dirs)
    taps = np.zeros((4, 5, 64, 256), f32)
    for s in range(-2, 3):
        taps[0, s + 2] = np.asarray(hconv_w)[:, :, 0, s + 2]
        taps[1, s + 2] = np.asarray(wconv_w)[:, :, s + 2, 0]
        taps[2, s + 2] = np.asarray(d19_w)[:, :, s + 2, 0]
        taps[3, s + 2] = np.asarray(d37_w)[:, :, 0, 2 - s]
    wbr = np.zeros((2, 128, 20, 64), f32)
    for kk in range(2):
        for idx in range(20):
            dd, ss = idx // 5, idx % 5
            wbr[kk, :, idx, :] = taps[dd, ss, :, kk * 128:(kk + 1) * 128].T
    bbr = np.stack([hconv_b, wconv_b, d19_b, d37_b], axis=1).astype(f32)

    return dict(
        w1t=w1t, b1f=b1f, wbr=wbr, bbr=bbr,
        winT=np.asarray(Win).T.astype(f32).copy(),
        bin2=np.stack([bin_[0:128], bin_[128:256]], axis=1).astype(f32),
        convw=np.asarray(convw)[:, 0, :].astype(f32).copy(),
        convb=np.asarray(convb).reshape(128, 1).astype(f32),
        nconvb=(-np.asarray(convb).reshape(128, 1)).astype(f32),
        wxT=_wxt64(Wx),
        wdtT=np.asarray(Wdt).T.astype(f32).copy(),
        bdt=np.asarray(bdt).reshape(128, 1).astype(f32),
        acols=(-np.exp(np.asarray(Alog))).astype(f32),
        dskip=np.asarray(Dskip).reshape(128, 1).astype(f32),
        woutT=np.asarray(Wout).T.astype(f32).copy(),
        bout=np.asarray(bout).reshape(64, 1).astype(f32),
    )




def _make_runner(nc):
    """Persistent jitted SPMD runner (mirrors bass2jax.run_bass_via_pjrt but
    caches the jitted callable and device-resident weight shards across calls)."""
    import jax
    import jax.numpy as jnp
    from jax.sharding import Mesh, PartitionSpec
    from jax.experimental.shard_map import shard_map
    from concourse import bass2jax, mybir as _mb
    bass2jax.install_neuronx_cc_hook()

    n_cores = 8
    in_names, out_names, out_avals, zero_outs = [], [], [], []
    partition_name = nc.partition_id_tensor.name if nc.partition_id_tensor else None
    for alloc in nc.m.functions[0].allocations:
        if not isinstance(alloc, _mb.MemoryLocationSet):
            continue
        name = alloc.memorylocations[0].name
        if alloc.kind == "ExternalInput":
            if name != partition_name:
                in_names.append(name)
        elif alloc.kind == "ExternalOutput":
            shape = tuple(alloc.tensor_shape)
            dtype = _mb.dt.np(alloc.dtype)
            out_names.append(name)
            out_avals.append(jax.core.ShapedArray(shape, dtype))
            zero_outs.append(np.zeros(shape, dtype))
    n_params = len(in_names)
    all_names = list(in_names) + list(out_names)
    if partition_name is not None:
        all_names.append(partition_name)

    def _body(*args):
        operands = list(args)
        if partition_name is not None:
            operands.append(bass2jax.partition_id_tensor())
        outs = bass2jax._bass_exec_p.bind(
            *operands, out_avals=tuple(out_avals), in_names=tuple(all_names),
            out_names=tuple(out_names), lowering_input_output_aliases=(),
            sim_require_finite=True, sim_require_nnan=True, nc=nc)
        return tuple(outs)

    devices = jax.devices()[:n_cores]
    mesh = Mesh(np.asarray(devices), ("core",))
    nin = n_params + len(out_names)
    sharded = jax.jit(shard_map(
        _body, mesh=mesh, in_specs=(PartitionSpec("core"),) * nin,
        out_specs=(PartitionSpec("core"),) * len(out_names), check_rep=False))

    _CACHE["sharded_fn"] = sharded

    def run(in_maps):
        concat_in = [np.concatenate([np.asarray(in_maps[c][nm])
                                     for c in range(n_cores)], axis=0)
                     for nm in in_names]
        concat_zeros = [np.zeros((n_cores * z.shape[0], *z.shape[1:]), z.dtype)
                        for z in zero_outs]
        out_arrs = sharded(*concat_in, *concat_zeros)
        out_arrs = [np.asarray(a) for a in out_arrs]
        return [{nm: out_arrs[i].reshape(n_cores, *out_avals[i].shape)[c]
                 for i, nm in enumerate(out_names)} for c in range(n_cores)]

    return run


def get_nc():
    if "nc" not in _CACHE:
        _CACHE["nc"] = _build_nc()
    return _CACHE["nc"]


def kernel(x, x2, **kw):
    nc = get_nc()
    wts = _prep_weights(**kw)
    xf = np.asarray(x, np.float32).reshape(8, CH, L)
    x2f = np.asarray(x2, np.float32).reshape(8, CH, L)
    in_maps = []
    for b in range(8):
        m = dict(wts)
        m["xb"] = np.ascontiguousarray(xf[b])
        m["x2b"] = np.ascontiguousarray(x2f[b])
        in_maps.append(m)
    if "runner" not in _CACHE:
        try:
            _CACHE["runner"] = _make_runner(nc)
        except Exception:
            _CACHE["runner"] = None
    if _CACHE["runner"] is not None:
        results = _CACHE["runner"](in_maps)
    else:
        results = run_bass_kernel_spmd(nc, in_maps, core_ids=list(range(8))).results
    out = np.stack([results[b]["out"] for b in range(8)], axis=0)
    return out.reshape(8, CH, H, W).astype(np.float32)


# revision 26
# speedup vs baseline: 72.7916x; 1.6524x over previous
"""Trainium2 Bass kernel for nn_Directionalmamba (B=8, CH=256, H=W=64).

Sharding: data-parallel over batch — each of the 8 NeuronCores runs one batch
element end-to-end (1x1 conv + BN/ReLU front-end, 4 directional selective
scans, 4 directional 5-tap conv branches, output assembly). No collectives.

Key mappings per core:
  - 1x1 conv / all projections: fp32r matmuls (F=512 chunks, PSUM accum).
  - Directional orders produced via layout tricks: transposed eviction for
    dir1; row-doubled layout for the two diagonal dirs so the (i+j)%64 /
    (j-i)%64 gathers become affine access patterns.
  - Conv branches = 5 shifted-AP matmuls on a zero-padded SBUF tile.
  - Selective scan: hardware prefix scan (tensor_tensor_scan) per (n, chunk)
    on GpSimd; a=exp(dt*A[:,n]) on ACT via per-partition scale; b=u*B with
    B/C broadcast through DMA-replicated (t,n)-interleaved bf16 rows;
    y=sum_n h*C via a masked cumulative scan (segment sum).
"""
import numpy as np

import concourse.bass as bass
import concourse.tile as tile
from concourse import mybir, bacc
from concourse.bass_utils import run_bass_kernel_spmd

F32 = mybir.dt.float32
F32R = mybir.dt.float32r
BF16 = mybir.dt.float16
AOT = mybir.AluOpType
ACTF = mybir.ActivationFunctionType

CH = 256
D, E, N, DTR, KCONV = 64, 128, 16, 4, 4
H = W = 64
L = H * W              # 4096
FC = 512               # matmul moving-dim chunk
NFC = L // FC          # 8
TC = 256               # scan time chunk
NTC = L // TC          # 16
PADW = 68
EPS = 1e-5

_CACHE = {}


def _m_ap(v, dims, extra_offset=0, keep_partition=True):
    """Manual access pattern: replace free dims of AP `v` with `dims`
    ([step, count] pairs, arbitrary steps) at `extra_offset` elements."""
    w = v.copy()
    w.offset = v.offset + extra_offset
    lead = [list(v.ap[0])] if keep_partition else []
    w.ap = mybir.VecI64Pair(lead + [list(d) for d in dims])
    return w


def _build_nc():
    nc = bacc.Bacc("TRN2", target_bir_lowering=False, debug=False)
    ap = {}

    def din(name, shape, dt=F32):
        ap[name] = nc.dram_tensor(name, list(shape), dt, kind="ExternalInput").ap()

    din("xb", (CH, L), F32R)
    din("x2b", (CH, L), F32R)
    din("w1t", (4, 128, 2, 128), F32R)   # [kk][k][m][j]: lhsT for 1x1 (BN-folded)
    din("b1f", (128, 2))
    din("wbr", (2, 128, 20, 64), F32R)   # [kk][k][dir*5+tap][o]
    din("bbr", (64, 4))
    din("winT", (64, 256), F32R)
    din("bin2", (128, 2))
    din("convw", (128, 4))
    din("convb", (128, 1))
    din("nconvb", (128, 1))
    din("wxT", (128, 64), F32R)
    din("wdtT", (4, 128), F32R)
    din("bdt", (128, 1))
    din("acols", (128, 16))
    din("dskip", (128, 1))
    din("woutT", (128, 64), F32R)
    din("bout", (64, 1))

    out_ap = nc.dram_tensor("out", [CH, L], F32, kind="ExternalOutput").ap()
    xc01 = nc.dram_tensor("xc01", [128, L], F32R).ap()
    xcdbl = nc.dram_tensor("xcdbl", [128, 2 * L], F32R).ap()
    cbr = nc.dram_tensor("cbr", [4, 64, L], F32).ap()
    bcp = nc.dram_tensor("bcp", [4, 2, 16 * L], BF16).ap()
    md = nc.dram_tensor("md", [4, 64, 2 * L], F32).ap()

    with tile.TileContext(nc) as tc:
        _body(tc, ap, out_ap, xc01, xcdbl, cbr, bcp, md)
    nc.compile()
    return nc


def _body(tc, ap, out_ap, xc01, xcdbl, cbr, bcp, md):
    nc = tc.nc
    with nc.allow_low_precision(reason="f32r tags are byte-identical to f32"), \
         tc.tile_pool(name="wts", bufs=1) as wpool:
        _body2(tc, wpool, ap, out_ap, xc01, xcdbl, cbr, bcp, md)


def _body2(tc, wpool, ap, out_ap, xc01, xcdbl, cbr, bcp, md):
    nc = tc.nc

    def wtile(name, shape, dt=F32):
        t = wpool.tile(list(shape), dt, name=name)
        nc.sync.dma_start(t[:], ap[name])
        return t

    winT = wtile("winT", (64, 256), F32R)
    bin2 = wtile("bin2", (128, 2))
    convw = wtile("convw", (128, 4))
    convb = wtile("convb", (128, 1))
    nconvb = wtile("nconvb", (128, 1))
    wxT = wtile("wxT", (128, 64), F32R)
    wdtT = wtile("wdtT", (4, 128), F32R)
    bdt = wtile("bdt", (128, 1))
    acols = wtile("acols", (128, 16))
    dskip = wtile("dskip", (128, 1))
    woutT = wtile("woutT", (128, 64), F32R)
    bout = wtile("bout", (64, 1))
    b1f = wtile("b1f", (128, 2))
    bbr = wtile("bbr", (64, 4))

    # ================= PHASE A: 1x1 conv + BN/ReLU + branches =================
    with tc.tile_pool(name="phA", bufs=1) as pa, \
         tc.tile_pool(name="phAp", bufs=3, space="PSUM") as pap:
        w1t = pa.tile([128, 4, 2, 128], F32R)
        nc.sync.dma_start(w1t[:], ap["w1t"].rearrange("a k b m -> k a b m"))
        wbr = pa.tile([128, 2, 20, 64], F32R)
        nc.sync.dma_start(wbr[:], ap["wbr"].rearrange("a k c m -> k a c m"))

        xk = []
        for i, (src, half) in enumerate([("xb", 0), ("xb", 1), ("x2b", 0), ("x2b", 1)]):
            t = pa.tile([128, L], F32R, tag=f"xk{i}", name=f"xk{i}")
            nc.sync.dma_start(t[:], ap[src][128 * half:128 * (half + 1), :])
            xk.append(t)

        pads = [pa.tile([128, PADW * PADW], F32R, tag=f"pad{i}", name=f"pad{i}")
                for i in range(2)]
        nc.vector.memset(pads[0][:].bitcast(F32), 0.0)
        nc.vector.memset(pads[1][:].bitcast(F32), 0.0)
        xc01_sb = pa.tile([128, L], F32R)
        xcdbl_sb = pa.tile([128, 2 * L], F32R)

        for m in range(2):
            for fc in range(NFC):
                ps = pap.tile([128, FC], F32, tag="ps1x1")
                for kk in range(4):
                    nc.tensor.matmul(
                        ps[:], w1t[:, kk, m, :].bitcast(F32R),
                        xk[kk][:, fc * FC:(fc + 1) * FC].bitcast(F32R),
                        start=(kk == 0), stop=(kk == 3))
                ps3 = ps[:].rearrange("p (i j) -> p i j", i=8, j=64)
                padv = pads[m][:].rearrange("p (r c) -> p r c", r=PADW, c=PADW)
                nc.scalar.activation(
                    padv[:, 2 + 8 * fc:2 + 8 * fc + 8, 2:66], ps3,
                    ACTF.Relu, bias=b1f[:, m:m + 1])
                if m == 0:
                    nc.scalar.activation(
                        xc01_sb[0:64, fc * FC:(fc + 1) * FC], ps[0:64, :],
                        ACTF.Relu, bias=b1f[0:64, 0:1])
                    tr = xc01_sb[64:128, :].rearrange(
                        "p (j i) -> p i j", j=64, i=64)[:, 8 * fc:8 * fc + 8, :]
                    nc.scalar.activation(tr, ps3[64:128], ACTF.Relu,
                                         bias=b1f[64:128, 0:1])
                else:
                    dblv = xcdbl_sb[:].rearrange("p (r c) -> p r c", r=64, c=128)
                    nc.scalar.activation(
                        dblv[:, 8 * fc:8 * fc + 8, 0:64], ps3,
                        ACTF.Relu, bias=b1f[:, 1:2])
                    nc.scalar.activation(
                        dblv[:, 8 * fc:8 * fc + 8, 64:128], ps3,
                        ACTF.Relu, bias=b1f[:, 1:2])
        nc.sync.dma_start(xc01, xc01_sb[:])
        nc.sync.dma_start(xcdbl, xcdbl_sb[:])

        # branches: taps (dr, dc): c1 (0,s) c2 (s,0) c3 (s,-s) c4 (s,-s)
        tap_dirs = [(0, 1), (1, 0), (1, -1), (1, -1)]
        cbr_sb = pa.tile([64, L], F32)
        for d in range(4):
            sr, sc = tap_dirs[d]
            for fc in range(NFC):
                psb = pap.tile([64, FC], F32, tag="psbr")
                first = True
                for s in range(-2, 3):
                    dr, dc = sr * s, sc * s
                    for kk in range(2):
                        rhs = pads[kk][:].rearrange(
                            "p (r c) -> p r c", r=PADW, c=PADW)[
                            :, 2 + 8 * fc + dr:2 + 8 * fc + dr + 8,
                            2 + dc:2 + dc + 64]
                        nc.tensor.matmul(
                            psb[:], wbr[:, kk, d * 5 + s + 2, :].bitcast(F32R),
                            rhs.bitcast(F32R),
                            start=first, stop=(s == 2 and kk == 1))
                        first = False
                nc.scalar.activation(
                    cbr_sb[:, fc * FC:(fc + 1) * FC], psb[:],
                    ACTF.Identity, bias=bbr[:, d:d + 1])
            nc.sync.dma_start(cbr[d], cbr_sb[:])

    # ================= PHASE B: 4 directional mamba sequences =================
    with tc.tile_pool(name="phB", bufs=1) as pb, \
         tc.tile_pool(name="phBr", bufs=2) as pbr, \
         tc.tile_pool(name="phBs", bufs=2) as pbs, \
         tc.tile_pool(name="phBp", bufs=4, space="PSUM") as pbp:
        for d in range(4):
            xi_pad = pb.tile([128, L + 32], F32R, tag="xi_pad")
            zs = pb.tile([128, L], F32, tag="zs")
            dt = pb.tile([128, L], F32, tag="dt")
            u = pb.tile([128, L], F32R, tag="u")
            dbl_sb = pb.tile([128, L], F32R, tag="dbl")
            bc16 = pb.tile([64, L], BF16, tag="bc16")

            nc.vector.memset(xi_pad[:, 0:3].bitcast(F32), 0.0)

            # ---- in-proj ----
            for fc in range(NFC):
                rr = pbr.tile([64, FC], F32R, tag="rhs", bufs=4)
                if d == 0:
                    nc.sync.dma_start(rr[:], xc01[0:64, fc * FC:(fc + 1) * FC])
                elif d == 1:
                    nc.sync.dma_start(rr[:], xc01[64:128, fc * FC:(fc + 1) * FC])
                elif d == 2:
                    src = _m_ap(xcdbl[0:64, :], [[129, 8], [1, 64]], 129 * 8 * fc)
                    nc.sync.dma_start(rr[:], src)
                else:
                    src = _m_ap(xcdbl[64:128, :], [[127, 8], [1, 64]],
                                64 + 127 * 8 * fc)
                    nc.sync.dma_start(rr[:], src)
                pxi = pbp.tile([128, FC], F32, tag="psB", name="pxi")
                nc.tensor.matmul(pxi[:], winT[:, 0:128].bitcast(F32R),
                                 rr[:].bitcast(F32R), start=True, stop=True)
                nc.scalar.activation(xi_pad[:, 3 + fc * FC:3 + (fc + 1) * FC],
                                     pxi[:], ACTF.Identity, bias=bin2[:, 0:1])
                pz = pbp.tile([128, FC], F32, tag="psB", name="pz")
                nc.tensor.matmul(pz[:], winT[:, 128:256].bitcast(F32R),
                                 rr[:].bitcast(F32R), start=True, stop=True)
                nc.scalar.activation(zs[:, fc * FC:(fc + 1) * FC], pz[:],
                                     ACTF.Identity, bias=bin2[:, 1:2])

            # ---- causal depthwise conv1d + SiLU -> u holds xic ----
            nc.vector.tensor_scalar(u[:], xi_pad[:, 0:L], convw[:, 0:1],
                                    None, AOT.mult)
            for k in range(1, 4):
                nc.vector.scalar_tensor_tensor(
                    u[:], xi_pad[:, k:k + L], convw[:, k:k + 1], u[:],
                    AOT.mult, AOT.add)
            # silu(u + convb) = (u+convb) * 1/(1+exp(-(u+convb)))
            nc.scalar.activation(dt[:], u[:], ACTF.Exp, scale=-1.0,
                                 bias=nconvb[:, 0:1])
            nc.vector.tensor_scalar(dt[:], dt[:], 1.0, None, AOT.add)
            nc.vector.reciprocal(dt[:], dt[:])
            nc.vector.scalar_tensor_tensor(u[:], u[:], convb[:, 0:1], dt[:],
                                           AOT.add, AOT.mult)

            # ---- dbl projection (dt_raw | B | C), dt projection ----
            for fc in range(NFC):
                pdb = pbp.tile([64, FC], F32, tag="psB", name="pdb")
                nc.tensor.matmul(pdb[:], wxT[:].bitcast(F32R),
                                 u[:, fc * FC:(fc + 1) * FC].bitcast(F32R),
                                 start=True, stop=True)
                nc.scalar.activation(dbl_sb[0:64, fc * FC:(fc + 1) * FC],
                                     pdb[:], ACTF.Copy)
            for fc in range(NFC):
                pdt = pbp.tile([128, FC], F32, tag="psB", name="pdt")
                nc.tensor.matmul(pdt[:], wdtT[:].bitcast(F32R),
                                 dbl_sb[0:4, fc * FC:(fc + 1) * FC].bitcast(F32R),
                                 start=True, stop=True)
                nc.scalar.activation(dt[:, fc * FC:(fc + 1) * FC], pdt[:],
                                     ACTF.Exp, bias=bdt[:, 0:1])
                nc.scalar.activation(dt[:, fc * FC:(fc + 1) * FC],
                                     dt[:, fc * FC:(fc + 1) * FC],
                                     ACTF.Ln, bias=1.0)

            # g = (xic * Dskip) * zs stored into xi_pad[:, 0:L] (xi_pad dead)
            nc.vector.scalar_tensor_tensor(xi_pad[:, 0:L], u[:], dskip[:, 0:1],
                                           zs[:], AOT.mult, AOT.mult)
            # u := dt * xic (in place; after dbl used xic)
            nc.vector.tensor_tensor(u[:], u[:], dt[:], AOT.mult)

            # export B,C as bf16 (t,n)-interleaved rows to DRAM
            nc.scalar.activation(bc16[32:64, :], dbl_sb[32:64, :], ACTF.Copy)
            for bi in range(2):
                dst = _m_ap(bcp[d, bi, :], [[1, 16], [16, L]],
                            keep_partition=False)
                nc.sync.dma_start(dst, bc16[32 + 16 * bi:48 + 16 * bi, :])

            # zs := silu(zs) using exp/recip (dbl_sb rows free as scratch)
            nc.scalar.activation(dbl_sb[:], zs[:], ACTF.Exp, scale=-1.0)
            nc.vector.tensor_scalar(dbl_sb[:], dbl_sb[:], 1.0, None, AOT.add)
            nc.vector.reciprocal(dbl_sb[:], dbl_sb[:])
            nc.gpsimd.tensor_tensor(zs[:], zs[:], dbl_sb[:], AOT.mult)

            # ---- scan chunks (software-pipelined y-stage) ----
            h_prev = None
            c_prev = None
            for c in range(NTC + 1):
                if c < NTC:
                    b_all = pbs.tile([128, 16 * TC], F32, tag="b_all")
                    brep = pbs.tile([128, 16 * TC], BF16, tag="brep", bufs=1)
                    nc.sync.dma_start(
                        brep[:], bcp[d, 0, 16 * TC * c:16 * TC * (c + 1)]
                        .partition_broadcast(128))
                    crep = pbs.tile([128, 16 * TC], BF16, tag="crep")
                    nc.sync.dma_start(
                        crep[:], bcp[d, 1, 16 * TC * c:16 * TC * (c + 1)]
                        .partition_broadcast(128))
                    u_bc = _m_ap(u[:], [[1, TC], [0, 16]], TC * c)
                    nc.gpsimd.tensor_tensor(
                        b_all[:].rearrange("p (t n) -> p t n", n=16),
                        u_bc, brep[:].rearrange("p (t n) -> p t n", n=16),
                        AOT.mult)
                    h_all = pbs.tile([128, 16 * TC], F32, tag="h_all")
                    hv = h_all[:].rearrange("p (t n) -> p t n", n=16)
                    bv = b_all[:].rearrange("p (t n) -> p t n", n=16)
                    for n in range(16):
                        a_n = pbs.tile([128, TC], F32, tag="a_n", bufs=4)
                        nc.scalar.activation(a_n[:], dt[:, TC * c:TC * (c + 1)],
                                             ACTF.Exp, scale=acols[:, n:n + 1])
                        if c == 0:
                            init = 0.0
                        else:
                            init = h_prev[:].rearrange(
                                "p (t n) -> p t n", n=16)[:, TC - 1:TC, n]
                        nc.vector.tensor_tensor_scan(
                            hv[:, :, n], a_n[:], bv[:, :, n], init,
                            AOT.mult, AOT.add)
                if c > 0:
                    cm1 = c - 1
                    nc.vector.tensor_tensor(h_prev[:], h_prev[:],
                                            c_prev[:], AOT.mult)
                    pv = h_prev[:].rearrange("p (t n) -> p t n", n=16)
                    for half in (8, 4, 2, 1):
                        nc.gpsimd.tensor_tensor(pv[:, :, 0:half],
                                                pv[:, :, 0:half],
                                                pv[:, :, half:2 * half], AOT.add)
                    tmp = pbs.tile([128, TC], F32, tag="a_n", bufs=4)
                    nc.vector.tensor_tensor(tmp[:], pv[:, :, 0],
                                            zs[:, TC * cm1:TC * c], AOT.mult)
                    nc.gpsimd.tensor_tensor(
                        xi_pad[:, TC * cm1:TC * c], tmp[:],
                        xi_pad[:, TC * cm1:TC * c], AOT.add)
                if c < NTC:
                    h_prev = h_all
                    c_prev = crep

            # ---- out-proj ----
            o_sb = pb.tile([64, L], F32, tag="o_sb")
            for fc in range(NFC):
                po = pbp.tile([64, FC], F32, tag="psB", name="po")
                nc.tensor.matmul(po[:], woutT[:].bitcast(F32R),
                                 xi_pad[:, fc * FC:(fc + 1) * FC].bitcast(F32R),
                                 start=True, stop=True)
                nc.scalar.activation(o_sb[:, fc * FC:(fc + 1) * FC], po[:],
                                     ACTF.Identity, bias=bout[:, 0:1])
            nc.sync.dma_start(md[d][:, 0:L], o_sb[:])
            if d >= 2:
                nc.sync.dma_start(md[d][:, L:2 * L], o_sb[:])

    # ================= PHASE C: assembly =================
    with tc.tile_pool(name="phC", bufs=2) as pc:
        for d in range(4):
            mo = pc.tile([64, 2 * L], F32, tag="mo")
            if d < 2:
                nc.sync.dma_start(mo[:, 0:L], md[d][:, 0:L])
            else:
                nc.sync.dma_start(mo[:], md[d])
            cb = pc.tile([64, L], F32, tag="cb")
            nc.sync.dma_start(cb[:], cbr[d])
            ofin = pc.tile([64, L], F32, tag="ofin")
            if d == 0:
                src = mo[:, 0:L]
            elif d == 1:
                src = _m_ap(mo[:], [[1, 64], [64, 64]])
            elif d == 2:
                src = _m_ap(mo[:], [[-63, 64], [64, 64]], L)
            else:
                src = _m_ap(mo[:], [[65, 64], [64, 64]])
            nc.vector.tensor_tensor(ofin[:], src, cb[:], AOT.add)
            nc.sync.dma_start(out_ap[64 * d:64 * (d + 1), :], ofin[:])




def _wxt64(Wx):
    wt = np.asarray(Wx).T.astype(np.float32)  # (128, 36)
    out = np.zeros((128, 64), np.float32)
    out[:, 0:4] = wt[:, 0:4]
    out[:, 32:48] = wt[:, 4:20]
    out[:, 48:64] = wt[:, 20:36]
    return out

def _prep_weights(w1, b1, bn_g, bn_b, bn_m, bn_v,
                  hconv_w, hconv_b, wconv_w, wconv_b, d19_w, d19_b, d37_w,
                  d37_b, Win, bin_, convw, convb, Wx, Wdt, bdt, Alog, Dskip,
                  Wout, bout):
    f32 = np.float32
    scale = (bn_g / np.sqrt(bn_v + EPS)).astype(f32)
    w1f = (np.asarray(w1)[:, :, 0, 0] * scale[:, None]).astype(f32)  # (256, 512)
    b1fv = ((np.asarray(b1) - bn_m) * scale + bn_b).astype(f32)

    w1t = np.zeros((4, 128, 2, 128), f32)
    for kk in range(4):
        for m in range(2):
            w1t[kk, :, m, :] = w1f[m * 128:(m + 1) * 128,
                                   kk * 128:(kk + 1) * 128].T
    b1f = np.stack([b1fv[0:128], b1fv[128:256]], axis=1)

    # branch taps: weight[s] for offset pattern (see _body tap_dirs)
    taps = np.zeros((4, 5, 64, 256), f32)
    for s in range(-2, 3):
        taps[0, s + 2] = np.asarray(hconv_w)[:, :, 0, s + 2]
        taps[1, s + 2] = np.asarray(wconv_w)[:, :, s + 2, 0]
        taps[2, s + 2] = np.asarray(d19_w)[:, :, s + 2, 0]
        taps[3, s + 2] = np.asarray(d37_w)[:, :, 0, 2 - s]
    wbr = np.zeros((2, 128, 20, 64), f32)
    for kk in range(2):
        for idx in range(20):
            dd, ss = idx // 5, idx % 5
            wbr[kk, :, idx, :] = taps[dd, ss, :, kk * 128:(kk + 1) * 128].T
    bbr = np.stack([hconv_b, wconv_b, d19_b, d37_b], axis=1).astype(f32)

    return dict(
        w1t=w1t, b1f=b1f, wbr=wbr, bbr=bbr,
        winT=np.asarray(Win).T.astype(f32).copy(),
        bin2=np.stack([bin_[0:128], bin_[128:256]], axis=1).astype(f32),
        convw=np.asarray(convw)[:, 0, :].astype(f32).copy(),
        convb=np.asarray(convb).reshape(128, 1).astype(f32),
        nconvb=(-np.asarray(convb).reshape(128, 1)).astype(f32),
        wxT=_wxt64(Wx),
        wdtT=np.asarray(Wdt).T.astype(f32).copy(),
        bdt=np.asarray(bdt).reshape(128, 1).astype(f32),
        acols=(-np.exp(np.asarray(Alog))).astype(f32),
        dskip=np.asarray(Dskip).reshape(128, 1).astype(f32),
        woutT=np.asarray(Wout).T.astype(f32).copy(),
        bout=np.asarray(bout).reshape(64, 1).astype(f32),
    )




def _make_runner(nc):
    """Persistent jitted SPMD runner (mirrors bass2jax.run_bass_via_pjrt but
    caches the jitted callable and device-resident weight shards across calls)."""
    import jax
    import jax.numpy as jnp
    from jax.sharding import Mesh, PartitionSpec
    from jax.experimental.shard_map import shard_map
    from concourse import bass2jax, mybir as _mb
    bass2jax.install_neuronx_cc_hook()

    n_cores = 8
    in_names, out_names, out_avals, zero_outs = [], [], [], []
    partition_name = nc.partition_id_tensor.name if nc.partition_id_tensor else None
    for alloc in nc.m.functions[0].allocations:
        if not isinstance(alloc, _mb.MemoryLocationSet):
            continue
        name = alloc.memorylocations[0].name
        if alloc.kind == "ExternalInput":
            if name != partition_name:
                in_names.append(name)
        elif alloc.kind == "ExternalOutput":
            shape = tuple(alloc.tensor_shape)
            dtype = _mb.dt.np(alloc.dtype)
            out_names.append(name)
            out_avals.append(jax.core.ShapedArray(shape, dtype))
            zero_outs.append(np.zeros(shape, dtype))
    n_params = len(in_names)
    all_names = list(in_names) + list(out_names)
    if partition_name is not None:
        all_names.append(partition_name)

    def _body(*args):
        operands = list(args)
        if partition_name is not None:
            operands.append(bass2jax.partition_id_tensor())
        outs = bass2jax._bass_exec_p.bind(
            *operands, out_avals=tuple(out_avals), in_names=tuple(all_names),
            out_names=tuple(out_names), lowering_input_output_aliases=(),
            sim_require_finite=True, sim_require_nnan=True, nc=nc)
        return tuple(outs)

    devices = jax.devices()[:n_cores]
    mesh = Mesh(np.asarray(devices), ("core",))
    nin = n_params + len(out_names)
    sharded = jax.jit(shard_map(
        _body, mesh=mesh, in_specs=(PartitionSpec("core"),) * nin,
        out_specs=(PartitionSpec("core"),) * len(out_names), check_rep=False))

    _CACHE["sharded_fn"] = sharded

    def run(in_maps):
        concat_in = [np.concatenate([np.asarray(in_maps[c][nm])
                                     for c in range(n_cores)], axis=0)
                     for nm in in_names]
        concat_zeros = [np.zeros((n_cores * z.shape[0], *z.shape[1:]), z.dtype)
                        for z in zero_outs]
        out_arrs = sharded(*concat_in, *concat_zeros)
        out_arrs = [np.asarray(a) for a in out_arrs]
        return [{nm: out_arrs[i].reshape(n_cores, *out_avals[i].shape)[c]
                 for i, nm in enumerate(out_names)} for c in range(n_cores)]

    return run


def get_nc():
    if "nc" not in _CACHE:
        _CACHE["nc"] = _build_nc()
    return _CACHE["nc"]


def kernel(x, x2, **kw):
    nc = get_nc()
    wts = _prep_weights(**kw)
    xf = np.asarray(x, np.float32).reshape(8, CH, L)
    x2f = np.asarray(x2, np.float32).reshape(8, CH, L)
    in_maps = []
    for b in range(8):
        m = dict(wts)
        m["xb"] = np.ascontiguousarray(xf[b])
        m["x2b"] = np.ascontiguousarray(x2f[b])
        in_maps.append(m)
    if "runner" not in _CACHE:
        try:
            _CACHE["runner"] = _make_runner(nc)
        except Exception:
            _CACHE["runner"] = None
    if _CACHE["runner"] is not None:
        results = _CACHE["runner"](in_maps)
    else:
        results = run_bass_kernel_spmd(nc, in_maps, core_ids=list(range(8))).results
    out = np.stack([results[b]["out"] for b in range(8)], axis=0)
    return out.reshape(8, CH, H, W).astype(np.float32)
